# revision 46
# baseline (speedup 1.0000x reference)
"""Trainium2 Bass kernel for nn_EntityMentionAggregation.

Reference computation (per batch b, M=N=16 mentions, H=768):
  self-attn over head mentions, cross-attn head->tail, sigmoid-gated fusion,
  mask-softmax pooling over mentions -> out [B, H].

Algebraic restructuring (exact, given the zero biases produced by
setup_inputs; nonzero projection biases fall back to numpy):
  s_scores = scale * head @ (Wsq^T Wsk) @ head^T          (A_s folded)
  c_scores = scale * head @ (Wcq^T Wck) @ tail^T          (A_c folded)
  out      = hpool @ Wsv^T + tpool
    hpool  = ws_s^T-weighted sum of head rows, ws_s = s_w^T (mw*gate/den_s)
    tpool  = ws_c^T-weighted sum of tail rows
  gate     = sigmoid(s_w@(head@u) + c_w@(tail@w2) + C0), u = Wsv^T Wg1
so the V projection runs on pooled vectors (16x fewer rows) and
self_out/cross_out are never materialized.

Precision split: the score path (big GEMM + packed per-tile attention
matmuls + gate dot-products) runs in fp8 e4m3 with DoubleRow perf mode
(2 k-tiles of 128 per matmul at 0.5 cyc/row); the value path (pooled
head/tail rows, final Wsv^T projection) stays fp16. The fp8 operands are
produced by a second SWDGE cast-load (f32->fp8) and transposed to
feature-major via the SBUF xbar with PAIRS of fp8 values packed in one
uint16 element; the resulting [feat-pair partition, 2, row] layout is
exactly DoubleRow's expected [K,2,N] k-tile shape (logical feature
f = 256c + 2p + i).  The folded A matrix is stored column-permuted
(per 256-block: even columns then odd columns) so the big GEMM's PSUM
partitions line up with the same pairing when its output chunks are used
as score-matmul weights.

Gate path: e = exp(scores/S) is transposed on the PE (identity matmul) so
den = e^T @ S*ones and gs = e^T @ (head@u) become 1-column matmuls,
removing the partition-broadcast DRAM round-trip of hv entirely.
Scores carry a factor S=32 (folded into A) so the fp8 tensors sit inside
e4m3's dynamic range; exp applies scale 1/S, and S cancels in
gate = sigmoid(gs/(S den) * S ...) via S-folded mw and S-valued ones.

Layout: batch is sharded 8 ways (512 batches/core); rows are processed in
tiles of 128 = 8 batches x 16 mentions. Cross-batch blocks are masked to
-inf via a rank-9 constant matmul and the per-tile key-mask via a rank-1
matmul, so softmax zeroes them exactly and the packed attention matrix is
block-diagonal -- which makes the pooling contractions plain matmuls
against block-diagonal weight columns built with a onehot multiply.
"""

import numpy as np
import ml_dtypes
import bass_rust
import concourse.bass as bass
import concourse.mybir as mybir
import concourse.tile as tile
from concourse.bass import ts
from concourse.bass_utils import run_bass_kernel_spmd

H = 768
B, M, N = 4096, 16, 16
NEG = -65504.0
P = 128
NCORES = 8
BC = B // NCORES          # batches per core = 512
ROWS = BC * M             # rows per core = 8192
TILES = ROWS // P         # 64 tiles (8 batches each)
ST = 4                    # tiles per supertile (GEMM moving N = 512)
NSUP = TILES // ST        # 16 supertiles
SN = ST * P               # 512 rows per supertile
GN = 512                  # GEMM moving width per PSUM pass (one bank)
KC = H // P               # 6 contraction chunks (128 each)
KC2 = H // (2 * P)        # 3 DoubleRow chunk pairs (256 each)
FO = 2 * H // P           # 12 score-feature chunks (A_s | A_c)
ACOLS = 2 * H + 1         # 1537: A_s | A_c | u
APAD = 1552               # ACOLS padded so the DoubleRow pair step is 16B-aligned
RPB = ROWS // M           # 512 pooled rows (batches) per core
S = 32.0                  # fp8 dynamic-range scale folded into A/u/w2/mw

F8 = mybir.dt.float8e4
F16 = mybir.dt.float16
F32 = mybir.dt.float32
U16 = mybir.dt.uint16
DR = mybir.MatmulPerfMode.DoubleRow
NP8 = ml_dtypes.float8_e4m3


def _split_sync_waits(nc):
    """Walrus caps sync waits per instruction (1 is the only universally
    accepted count in this toolchain). Hoist excess waits onto preceding
    single-wait EventSemaphore carriers on the same engine."""
    for f in nc.m.functions:
        for bb in f.blocks:
            il = bb.instructions
            new_il = []
            changed = False
            for inst in il:
                si = inst.sync_info
                if si is not None and len(si.on_wait) > 1:
                    waits = list(si.on_wait)
                    k = 0
                    while len(waits) > 1:
                        w, waits = waits[0], waits[1:]
                        d = bass_rust.InstEventSemaphore(
                            name=f"{inst.name}-wsplit{k}", ins=[], outs=[])
                        d.engine = inst.engine
                        d.sync_info = bass_rust.SyncInfo(on_wait=[w], on_update=[])
                        new_il.append(d)
                        k += 1
                        changed = True
                    inst.sync_info = bass_rust.SyncInfo(
                        on_wait=waits, on_update=list(si.on_update))
                new_il.append(inst)
            if changed:
                bb.instructions = new_il


def _build_nc(split=True):
    nc = bass.Bass(target_bir_lowering=False)

    head_d = nc.dram_tensor("head", [ROWS, H], F32, kind="ExternalInput")
    tail_d = nc.dram_tensor("tail", [ROWS, H], F32, kind="ExternalInput")
    acat_d = nc.dram_tensor("acat", [KC2, P, 2, APAD], F8, kind="ExternalInput")
    w2_d = nc.dram_tensor("w2c", [KC2, P, 2, 2], F8, kind="ExternalInput")
    wsvT_d = nc.dram_tensor("wsvT", [KC, P, H], F16, kind="ExternalInput")
    c9l_d = nc.dram_tensor("c9l", [9, P], F16, kind="ExternalInput")
    c9r_d = nc.dram_tensor("c9r", [9, P], F16, kind="ExternalInput")
    ones1_d = nc.dram_tensor("ones1", [1, P], F16, kind="ExternalInput")
    onesc_d = nc.dram_tensor("onesc", [P, 1], F16, kind="ExternalInput")
    onehot_d = nc.dram_tensor("onehot", [P, 8], F16, kind="ExternalInput")
    vs_d = nc.dram_tensor("vs", [1, TILES * P], F16, kind="ExternalInput")
    vc_d = nc.dram_tensor("vc", [1, TILES * P], F16, kind="ExternalInput")
    mw_d = nc.dram_tensor("mw", [P, TILES], F32, kind="ExternalInput")
    ident_d = nc.dram_tensor("ident", [P, P], F32, kind="ExternalInput")
    identh_d = nc.dram_tensor("identh", [P, P], F16, kind="ExternalInput")
    c0_d = nc.dram_tensor("c0", [P, 1], F32, kind="ExternalInput")
    out_d = nc.dram_tensor("out", [BC, H], F32, kind="ExternalOutput")

    with tile.TileContext(nc) as tc:
        _emit(nc, tc, head_d, tail_d, acat_d, w2_d, wsvT_d, c9l_d, c9r_d,
              ones1_d, onesc_d, onehot_d, vs_d, vc_d, mw_d, ident_d,
              identh_d, c0_d, out_d)
    if split:
        _split_sync_waits(nc)
    return nc


def _emit(nc, tc, head_d, tail_d, acat_d, w2_d, wsvT_d, c9l_d, c9r_d,
          ones1_d, onesc_d, onehot_d, vs_d, vc_d, mw_d, ident_d,
          identh_d, c0_d, out_d):
    from contextlib import ExitStack
    Exp = mybir.ActivationFunctionType.Exp
    Sig = mybir.ActivationFunctionType.Sigmoid
    Ident = mybir.ActivationFunctionType.Identity
    mult = mybir.AluOpType.mult
    ctx = ExitStack()
    with ctx:
        const = ctx.enter_context(tc.tile_pool(name="const", bufs=1))
        sup = ctx.enter_context(tc.tile_pool(name="sup", bufs=2))
        pt = ctx.enter_context(tc.tile_pool(name="pt", bufs=6))
        acc = ctx.enter_context(tc.tile_pool(name="acc", bufs=1))
        psg = ctx.enter_context(tc.tile_pool(name="psg", bufs=2, space="PSUM"))
        pss = ctx.enter_context(tc.tile_pool(name="pss", bufs=4, space="PSUM"))
        psw = ctx.enter_context(tc.tile_pool(name="psw", bufs=2, space="PSUM"))

        # ---- constants ----
        acat8 = const.tile([P, KC2, 2, APAD], F8)
        nc.sync.dma_start(out=acat8[:], in_=acat_d.rearrange("c p i m -> p c i m"))
        w2c8 = const.tile([P, KC2, 2, 2], F8)
        nc.sync.dma_start(out=w2c8[:], in_=w2_d.rearrange("c p i m -> p c i m"))
        c9l = const.tile([9, P], F16)
        nc.sync.dma_start(out=c9l[:], in_=c9l_d[:, :])
        c9r = const.tile([9, P], F16)
        nc.sync.dma_start(out=c9r[:], in_=c9r_d[:, :])
        ones1 = const.tile([1, P], F16)
        nc.sync.dma_start(out=ones1[:], in_=ones1_d[:, :])
        onesc = const.tile([P, 1], F16)
        nc.sync.dma_start(out=onesc[:], in_=onesc_d[:, :])
        onehot = const.tile([P, 8], F16)
        nc.sync.dma_start(out=onehot[:], in_=onehot_d[:, :])
        vs_all = const.tile([1, TILES * P], F16)
        nc.scalar.dma_start(out=vs_all[:], in_=vs_d[:, :])
        vc_all = const.tile([1, TILES * P], F16)
        nc.scalar.dma_start(out=vc_all[:], in_=vc_d[:, :])

        # ---- per-core accumulators ----
        hp_all = acc.tile([P, KC, RPB], F16)   # pooled head, feature-major
        tp_all = acc.tile([P, KC, RPB], F16)   # pooled tail, feature-major

        # loads are batched per PAIR of supertiles: the SWDGE descriptor-gen
        # time on the Pool engine is ~1-2.4us per instruction regardless of
        # size, and 4 cast-loads/supertile made Pool the DMA-issue serializer
        SG = 2 * ST
        head_r = head_d.rearrange("(g t p) h -> g p t h", t=SG, p=P)
        tail_r = tail_d.rearrange("(g t p) h -> g p t h", t=SG, p=P)

        loaded16 = {}
        loaded8 = {}

        def emit_loads16(g):
            # fp16 copies feed the value-side pools
            h16 = sup.tile([P, SG, H], F16, tag="h16", name=f"h16_{g}")
            t16 = sup.tile([P, SG, H], F16, tag="t16", name=f"t16_{g}")
            nc.gpsimd.dma_start(out=h16[:], in_=head_r[g])
            nc.gpsimd.dma_start(out=t16[:], in_=tail_r[g])
            loaded16[g] = (h16, t16)

        def emit_loads8(g):
            # fp8 copies feed the score-side GEMMs; cast directly from the
            # f32 rows by SWDGE
            h8 = sup.tile([P, SG, H], F8, tag="h8", name=f"h8_{g}")
            t8 = sup.tile([P, SG, H], F8, tag="t8", name=f"t8_{g}")
            nc.gpsimd.dma_start(out=h8[:], in_=head_r[g])
            nc.gpsimd.dma_start(out=t8[:], in_=tail_r[g])
            loaded8[g] = (h8, t8)

        transposed = {}

        def emit_transpose(s):
            # xbar transpose to feature-major with fp8 PAIRS packed in uint16
            h8, t8 = loaded8[s // 2]
            off = ST * (s % 2)
            hT = sup.tile([P, KC2, SN], U16, tag="hT", name=f"hT{s}", bufs=3)
            tT = sup.tile([P, KC2, SN], U16, tag="tT", name=f"tT{s}", bufs=3)
            for t in range(ST):
                nc.sync.dma_start_transpose(hT[:, :, ts(t, P)],
                                            h8[:, off + t, :].bitcast(U16))
                nc.sync.dma_start_transpose(tT[:, :, ts(t, P)],
                                            t8[:, off + t, :].bitcast(U16))
            transposed[s] = (hT, tT)

        def f8v(tT_, cc):
            # DoubleRow moving view of a pair-packed chunk: [K=128, 2, n]
            return tT_[:, cc, :].bitcast(F8).rearrange(
                "p (n two) -> p two n", two=2)

        out_fm = acc.tile([P, KC, RPB], F32)
        out_sb = acc.tile([P, BC // P, H], F32)
        out_r = out_d.rearrange("(r p) h -> p r h", p=P)

        def emit_final(half):
            # out = hpool @ Wsv^T + tpool for one half of the batches,
            # then transpose feature-major -> row-major and store.
            # Emitted per half so the first half overlaps the last supertile.
            bs = slice(half * (RPB // 2), (half + 1) * (RPB // 2))
            for j in range(KC):
                po_full = psg.tile([P, GN], F32, tag="pg", name=f"po{half}_{j}")
                po = po_full[:, :RPB // 2]
                for c in range(KC):
                    nc.tensor.matmul(po, wsvT[:, c, ts(j, P)], hp_all[:, c, bs],
                                     start=(c == 0), stop=(c == KC - 1))
                nc.vector.tensor_add(out=out_fm[:, j, bs], in0=po,
                                     in1=tp_all[:, j, bs])
            for r in range(half * (BC // P // 2), (half + 1) * (BC // P // 2)):
                for j in range(KC):
                    ptr_full = psg.tile([P, GN], F32, tag="pg", name=f"ptr{r}_{j}")
                    ptr = ptr_full[:, :P]
                    nc.tensor.transpose(ptr[:], out_fm[:, j, ts(r, P)], ident[:])
                    nc.scalar.copy(out_sb[:, r, ts(j, P)], ptr[:])
                nc.sync.dma_start(out=out_r[:, r, :], in_=out_sb[:, r, :])

        emit_loads16(0)
        emit_loads8(0)
        emit_transpose(0)
        emit_transpose(1)
        wsvT = const.tile([P, KC, H], F16)
        nc.sync.dma_start(out=wsvT[:], in_=wsvT_d.rearrange("c p m -> p c m"))
        mw_all = const.tile([P, TILES], F32)
        nc.sync.dma_start(out=mw_all[:], in_=mw_d[:, :])
        ident = const.tile([P, P], F32)
        nc.sync.dma_start(out=ident[:], in_=ident_d[:, :])
        identh = const.tile([P, P], F16)
        nc.sync.dma_start(out=identh[:], in_=identh_d[:, :])
        c0 = const.tile([P, 1], F32)
        nc.sync.dma_start(out=c0[:], in_=c0_d[:, :])
        hA8s = {}

        def emit_gemm(s):
            # -- big GEMM: hA = head @ [A_s | A_c], feature-major, fp8 DR --
            hT, tT = transposed[s]
            hA8 = sup.tile([P, FO, SN], F8, tag="hA8", name=f"hA8_{s}")
            for j in range(FO):
                for hh in range(SN // GN):
                    pg = psg.tile([P, GN], F32, tag="pg")
                    for cc in range(KC2):
                        nc.tensor.matmul(pg[:], acat8[:, cc, :, ts(j, P)],
                                         f8v(hT, cc)[:, :, ts(hh, GN)],
                                         start=(cc == 0),
                                         stop=(cc == KC2 - 1), perf_mode=DR)
                    if (2 * j + hh) % 24 < 14:
                        nc.scalar.copy(hA8[:, j, ts(hh, GN)], pg[:])
                    else:
                        nc.vector.tensor_copy(hA8[:, j, ts(hh, GN)], pg[:])
            hA8s[s] = hA8

        def emit_tiles(s_idx):
            h16g, t16g = loaded16[s_idx // 2]
            voff = ST * (s_idx % 2)
            hT, tT = transposed.pop(s_idx)
            hA8 = hA8s.pop(s_idx)
            h16 = h16g[:, voff:voff + ST, :]
            t16 = t16g[:, voff:voff + ST, :]

            def tile_body(t, tg):
                # -- packed scores (8 batches x 16x16) + masks --
                ps_pair = pss.tile([P, 2, P], F32, tag="ps")
                ps_s = ps_pair[:, 0, :]
                ps_c = ps_pair[:, 1, :]
                for cc in range(KC2):
                    nc.tensor.matmul(ps_s, hA8[:, 2 * cc:2 * cc + 2, ts(t, P)],
                                     f8v(hT, cc)[:, :, ts(t, P)],
                                     start=(cc == 0), stop=False, perf_mode=DR)
                nc.tensor.matmul(ps_s, c9l[:], c9r[:], start=False, stop=False)
                nc.tensor.matmul(ps_s, ones1[:], vs_all[:, ts(tg, P)],
                                 start=False, stop=True)
                for cc in range(KC2):
                    nc.tensor.matmul(ps_c,
                                     hA8[:, KC + 2 * cc:KC + 2 * cc + 2, ts(t, P)],
                                     f8v(tT, cc)[:, :, ts(t, P)],
                                     start=(cc == 0), stop=False, perf_mode=DR)
                nc.tensor.matmul(ps_c, c9l[:], c9r[:], start=False, stop=False)
                nc.tensor.matmul(ps_c, ones1[:], vc_all[:, ts(tg, P)],
                                 start=False, stop=True)

                # -- gate dot inputs: hv = head@u, tv = tail@w2 (key-major,
                # 1-column DoubleRow matmuls, ~free on the PE) --
                # wp cols 104:232 (f32) double as the fp16 e^T landing zone
                # via bitcast, so eT shares wp's PSUM bank.
                wp = psw.tile([P, 232], F32, tag="wp")
                # u and w2 are kept as fp8 hi+lo pairs (the gate is the
                # precision-dominant path); the extra 1-column matmuls are
                # free on the PE (cost scales with out free size).
                for k in range(4 * KC2):
                    cc, i, r = k // 4, (k // 2) % 2, k % 2
                    nc.tensor.matmul(wp[:, 102:103],
                                     f8v(hT, cc)[:, i, ts(t, P)],
                                     acat8[:, cc, i, 1536 + r:1537 + r],
                                     start=(k == 0), stop=(k == 4 * KC2 - 1))
                for k in range(4 * KC2):
                    cc, i, r = k // 4, (k // 2) % 2, k % 2
                    nc.tensor.matmul(wp[:, 103:104],
                                     f8v(tT, cc)[:, i, ts(t, P)],
                                     w2c8[:, cc, i, r:r + 1],
                                     start=(k == 0), stop=(k == 4 * KC2 - 1))

                # -- softmax numerators (free axis); scores are O(5) bounded
                # and masked lanes are ~-2e3 after the 1/S exp scale, so fp32
                # exp neither overflows nor loses the reference's exactness --
                e_pair = pt.tile([P, 2, P], F16, tag="e_pair", bufs=4)
                nc.scalar.activation(out=e_pair[:], in_=ps_pair[:], func=Exp,
                                     bias=0.0, scale=1.0 / S)

                # -- e^T on the PE so den/gs become 1-column matmuls --
                eT = wp[:, 104:232].bitcast(F16).rearrange(
                    "p (c n) -> p c n", c=2)
                nc.tensor.transpose(eT[:, 0, :], e_pair[:, 0, :], identh[:])
                nc.tensor.transpose(eT[:, 1, :], e_pair[:, 1, :], identh[:])
                esT = pt.tile([P, 2, P], F16, tag="esT", bufs=4)
                nc.vector.tensor_copy(esT[:], eT[:])
                hvtv = pt.tile([P, 2], F16, tag="hvtv", bufs=4)
                nc.vector.tensor_copy(hvtv[:], wp[:, 102:104])

                # cols: 98 gs_num, 99 gc_num, 100 S*den_s, 101 S*den_c
                nc.tensor.matmul(wp[:, 98:99], esT[:, 0, :], hvtv[:, 0:1],
                                 start=True, stop=True)
                nc.tensor.matmul(wp[:, 99:100], esT[:, 1, :], hvtv[:, 1:2],
                                 start=True, stop=True)
                nc.tensor.matmul(wp[:, 100:101], esT[:, 0, :], onesc[:],
                                 start=True, stop=True)
                nc.tensor.matmul(wp[:, 101:102], esT[:, 1, :], onesc[:],
                                 start=True, stop=True)

                rden = pt.tile([P, 2], F32, tag="rden")
                nc.vector.reciprocal(out=rden[:], in_=wp[:, 100:102])
                # gate = sigmoid(garg) computed as 1/(1+exp(-garg)) so the ACT
                # engine only ever uses the Exp table (Sigmoid lives in a
                # different act-table set and every switch costs a 1.3us table
                # load). The sign flips ride the host constants: onesc=-S and
                # mw=-S*mw make rden negative and a_s/a_c positive again.
                t1 = pt.tile([P, 1], F32, tag="t1")
                nc.scalar.activation(out=t1[:], in_=wp[:, 99:100], func=Ident,
                                     bias=c0[:, 0:1], scale=rden[:, 1:2])
                eg = pt.tile([P, 1], F32, tag="eg")
                nc.scalar.activation(out=eg[:], in_=wp[:, 98:99], func=Exp,
                                     bias=t1[:, 0:1], scale=rden[:, 0:1])
                gp = pt.tile([P, 1], F32, tag="gp")
                nc.vector.tensor_scalar_add(out=gp[:], in0=eg[:], scalar1=1.0)
                gate = pt.tile([P, 1], F32, tag="gate")
                nc.vector.reciprocal(out=gate[:], in_=gp[:])

                # -- pooling coefficient vectors (fold S*mw and 1/(S den)) --
                mwg = pt.tile([P, 1], F16, tag="mwg")       # S*mw*gate
                nc.vector.tensor_mul(out=mwg[:], in0=mw_all[:, tg:tg + 1],
                                     in1=gate[:])
                a_s = pt.tile([P, 1], F16, tag="a_s")
                nc.vector.tensor_mul(out=a_s[:], in0=mwg[:], in1=rden[:, 0:1])
                mwc = pt.tile([P, 1], F16, tag="mwc")       # S*mw*(1-gate)
                nc.vector.tensor_sub(out=mwc[:], in0=mw_all[:, tg:tg + 1],
                                     in1=mwg[:])
                a_c = pt.tile([P, 1], F16, tag="a_c")
                nc.vector.tensor_mul(out=a_c[:], in0=mwc[:], in1=rden[:, 1:2])

                # -- ws = e^T @ a : per-key pooled weights (block-diag safe) --
                nc.tensor.matmul(wp[:, 96:97], e_pair[:, 0, :], a_s[:],
                                 start=True, stop=True)
                nc.tensor.matmul(wp[:, 97:98], e_pair[:, 1, :], a_c[:],
                                 start=True, stop=True)

                # -- block-diagonal weight columns via onehot --
                diag_s = pt.tile([P, 8], F16, tag="diag_s")
                diag_c = pt.tile([P, 8], F16, tag="diag_c")
                nc.vector.tensor_tensor(out=diag_s[:],
                                        in0=wp[:, 96:97].to_broadcast([P, 8]),
                                        in1=onehot[:], op=mult)
                nc.vector.tensor_tensor(out=diag_c[:],
                                        in0=wp[:, 97:98].to_broadcast([P, 8]),
                                        in1=onehot[:], op=mult)

                # -- pools: feature-major pooled vectors for 8 batches --
                ps_hp = wp[:, 0:48].rearrange("p (c e) -> p c e", e=8)
                ps_tp = wp[:, 48:96].rearrange("p (c e) -> p c e", e=8)
                for c in range(KC):
                    nc.tensor.matmul(ps_hp[:, c, :], h16[:, t, ts(c, P)],
                                     diag_s[:], start=True, stop=True)
                    nc.tensor.matmul(ps_tp[:, c, :], t16[:, t, ts(c, P)],
                                     diag_c[:], start=True, stop=True)
                nc.vector.tensor_copy(hp_all[:, :, tg * 8:(tg + 1) * 8], ps_hp)
                nc.vector.tensor_copy(tp_all[:, :, tg * 8:(tg + 1) * 8], ps_tp)

            for t in range(ST):
                tile_body(t, s_idx * ST + t)

        for s_idx in range(NSUP):
            if s_idx == NSUP // 2:
                emit_final(0)
            if s_idx % 2 == 0 and s_idx // 2 + 1 < NSUP // 2:
                emit_loads16(s_idx // 2 + 1)
                emit_loads8(s_idx // 2 + 1)
            if s_idx + 2 < NSUP:
                emit_transpose(s_idx + 2)
            emit_gemm(s_idx)
            emit_tiles(s_idx)

        emit_final(1)


_NC_CACHE = None


def _get_nc():
    global _NC_CACHE
    if _NC_CACHE is None:
        _NC_CACHE = _build_nc()
    return _NC_CACHE


def _host_prep(Wsq, Wsk, Wsv, Wcq, Wck, Wg, bg, bsv,
               head_mask, tail_mask):
    """Fold weights; build per-core constant tensors (shared across cores
    except the mask-derived ones)."""
    f64 = np.float64
    scale = 1.0 / np.sqrt(f64(H))
    A_s = (Wsq.astype(f64).T @ Wsk.astype(f64)) * scale
    A_c = (Wcq.astype(f64).T @ Wck.astype(f64)) * scale
    A = np.concatenate([A_s, A_c], axis=1)                         # [768, 1536]
    # per 256-block of output features: even columns then odd columns, so
    # the big GEMM's PSUM chunk pairs (2c, 2c+1) hold features 256c+2p+i
    colperm = np.concatenate([
        np.concatenate([np.arange(256 * b, 256 * b + 256, 2),
                        np.arange(256 * b + 1, 256 * b + 256, 2)])
        for b in range(2 * H // 256)])
    Wg1 = Wg[0, :H].astype(f64)
    w2 = Wg[0, H:].astype(f64)
    u = Wsv.astype(f64).T @ Wg1
    uS = S * u
    u_hi = (uS.astype(np.float32)).astype(NP8)
    u_lo = uS - u_hi.astype(f64)                  # quantized again by the cast
    w2S = S * w2
    w2_hi = (w2S.astype(np.float32)).astype(NP8)
    w2_lo = w2S - w2_hi.astype(f64)
    acat = np.concatenate([A[:, colperm], u_hi.astype(f64)[:, None] / S,
                           u_lo[:, None] / S,
                           np.zeros((H, APAD - ACOLS - 1))], axis=1)  # [768, 1552]
    # rows (input features) interleaved: acat8[c, p, i] = S*acat[256c+2p+i]
    acat8 = (S * acat).reshape(KC2, P, 2, APAD).astype(NP8)
    w2_8 = np.stack([w2_hi.astype(f64), w2_lo], axis=-1)
    w2_8 = (w2_8).reshape(KC2, P, 2, 2).astype(NP8)
    wsvT_t = Wsv.astype(f64).T.reshape(KC, P, H).astype(np.float16)

    g = np.arange(P) // M                                          # group id per row
    c9l = np.zeros((9, P), np.float16)
    c9r = np.zeros((9, P), np.float16)
    c9l[0] = 1.0
    c9r[0] = NEG
    for k in range(8):
        c9l[1 + k] = (g == k).astype(np.float16)
        c9r[1 + k] = -NEG * (g == k).astype(np.float16)
    ones1 = np.ones((1, P), np.float16)
    onesc = np.full((P, 1), -S, np.float16)   # negative: see gate-as-exp note
    onehot = np.zeros((P, 8), np.float16)
    onehot[np.arange(P), g] = 1.0

    C0 = float(bg[0] + f64(bsv) @ Wg1)
    c0 = np.full((P, 1), -C0, np.float32)     # negated: gate-as-exp
    ident = np.eye(P, dtype=np.float32)
    identh = np.eye(P, dtype=np.float16)

    # per-core mask-derived tensors
    hm = head_mask.reshape(NCORES, BC, M)
    tm = tail_mask.reshape(NCORES, BC, N)
    vs, vc, mw = [], [], []
    for i in range(NCORES):
        vs.append(((1 - hm[i]).astype(np.float16) * np.float16(NEG))
                  .reshape(1, TILES * P))
        vc.append(((1 - tm[i]).astype(np.float16) * np.float16(NEG))
                  .reshape(1, TILES * P))
        e = np.exp(hm[i].astype(f64))
        mwi = (-S * e / e.sum(axis=1, keepdims=True)).astype(np.float32)
        mw.append(mwi.reshape(TILES, P).T.copy())                    # [P, TILES]
    shared = dict(acat=acat8, w2c=w2_8, wsvT=wsvT_t, c9l=c9l, c9r=c9r,
                  ones1=ones1, onesc=onesc, onehot=onehot, ident=ident,
                  identh=identh, c0=c0)
    return shared, vs, vc, mw


def _core_feeds(head_mentions, tail_mentions, shared, vs, vc, mw, i):
    hm = head_mentions.reshape(NCORES, ROWS, H)
    tm = tail_mentions.reshape(NCORES, ROWS, H)
    feeds = {"head": np.ascontiguousarray(hm[i]),
             "tail": np.ascontiguousarray(tm[i]),
             "vs": vs[i], "vc": vc[i], "mw": mw[i]}
    feeds.update(shared)
    return feeds


def _reference_numpy(head_mentions, tail_mentions, head_mask, tail_mask,
                     Wsq, bsq, Wsk, bsk, Wsv, bsv, Wcq, bcq, Wck, bck, Wg, bg):
    """Exact fallback (only used if projection biases are nonzero)."""
    f = np.float32
    scale = f(1.0) / np.sqrt(f(H))
    hm = head_mentions.astype(f)
    tm = tail_mentions.astype(f)
    sq = hm @ Wsq.T + bsq
    sk = hm @ Wsk.T + bsk
    sv = hm @ Wsv.T + bsv
    ss = np.einsum("bmh,bnh->bmn", sq, sk) * scale
    ss = np.where(head_mask[:, None, :] == 0, f(NEG), ss)
    ss = ss - ss.max(-1, keepdims=True)
    e = np.exp(ss)
    sw = e / e.sum(-1, keepdims=True)
    self_out = np.einsum("bmn,bnh->bmh", sw, sv)
    cq = hm @ Wcq.T + bcq
    ck = tm @ Wck.T + bck
    cs = np.einsum("bmh,bnh->bmn", cq, ck) * scale
    cs = np.where(tail_mask[:, None, :] == 0, f(NEG), cs)
    cs = cs - cs.max(-1, keepdims=True)
    ec = np.exp(cs)
    cw = ec / ec.sum(-1, keepdims=True)
    cross_out = np.einsum("bmn,bnh->bmh", cw, tm)
    gate_in = np.concatenate([self_out, cross_out], axis=-1)
    gate = 1.0 / (1.0 + np.exp(-(np.einsum("bmh,oh->bmo", gate_in, Wg) + bg)))
    fused = gate * self_out + (1 - gate) * cross_out
    mexp = np.exp(head_mask.astype(f))
    mw = (mexp / mexp.sum(1, keepdims=True))[:, :, None]
    return (fused * mw).sum(axis=1)


def kernel(head_mentions, tail_mentions, head_mask, tail_mask,
           Wsq, bsq, Wsk, bsk, Wsv, bsv, Wcq, bcq, Wck, bck, Wg, bg,
           _trace=False):
    head_mentions = np.asarray(head_mentions)
    tail_mentions = np.asarray(tail_mentions)
    head_mask = np.asarray(head_mask)
    tail_mask = np.asarray(tail_mask)
    args = dict(Wsq=np.asarray(Wsq), bsq=np.asarray(bsq), Wsk=np.asarray(Wsk),
                bsk=np.asarray(bsk), Wsv=np.asarray(Wsv), bsv=np.asarray(bsv),
                Wcq=np.asarray(Wcq), bcq=np.asarray(bcq), Wck=np.asarray(Wck),
                bck=np.asarray(bck), Wg=np.asarray(Wg), bg=np.asarray(bg))

    # The folded formulation absorbs bg/bsv exactly; nonzero Q/K-side biases
    # (never produced by this problem's setup) would change the softmax and
    # are handled by the exact numpy fallback.
    if any(np.any(args[k] != 0) for k in ("bsq", "bsk", "bcq", "bck")):
        return _reference_numpy(head_mentions, tail_mentions, head_mask,
                                tail_mask, **args).astype(np.float32)

    shared, vs, vc, mw = _host_prep(args["Wsq"], args["Wsk"], args["Wsv"],
                                    args["Wcq"], args["Wck"], args["Wg"],
                                    args["bg"], args["bsv"],
                                    head_mask, tail_mask)

    nc = _get_nc()
    in_maps = [_core_feeds(head_mentions, tail_mentions, shared, vs, vc, mw, i)
               for i in range(NCORES)]
    res = run_bass_kernel_spmd(nc, in_maps, core_ids=list(range(NCORES)),
                               trace=_trace)
    out = np.concatenate([res.results[i]["out"] for i in range(NCORES)], axis=0)
    if _trace:
        kernel._last_result = res
    return out.astype(np.float32)


# revision 56
# speedup vs baseline: 1.0049x; 1.0049x over previous
"""Trainium2 Bass kernel for nn_EntityMentionAggregation.

Reference computation (per batch b, M=N=16 mentions, H=768):
  self-attn over head mentions, cross-attn head->tail, sigmoid-gated fusion,
  mask-softmax pooling over mentions -> out [B, H].

Algebraic restructuring (exact, given the zero biases produced by
setup_inputs; nonzero projection biases fall back to numpy):
  s_scores = scale * head @ (Wsq^T Wsk) @ head^T          (A_s folded)
  c_scores = scale * head @ (Wcq^T Wck) @ tail^T          (A_c folded)
  out      = hpool @ Wsv^T + tpool
    hpool  = ws_s^T-weighted sum of head rows, ws_s = s_w^T (mw*gate/den_s)
    tpool  = ws_c^T-weighted sum of tail rows
  gate     = sigmoid(s_w@(head@u) + c_w@(tail@w2) + C0), u = Wsv^T Wg1
so the V projection runs on pooled vectors (16x fewer rows) and
self_out/cross_out are never materialized.

Precision split: the score path (big GEMM + packed per-tile attention
matmuls + gate dot-products) runs in fp8 e4m3 with DoubleRow perf mode
(2 k-tiles of 128 per matmul at 0.5 cyc/row); the value path (pooled
head/tail rows, final Wsv^T projection) stays fp16. The fp8 operands are
produced by a second SWDGE cast-load (f32->fp8) and transposed to
feature-major via the SBUF xbar with PAIRS of fp8 values packed in one
uint16 element; the resulting [feat-pair partition, 2, row] layout is
exactly DoubleRow's expected [K,2,N] k-tile shape (logical feature
f = 256c + 2p + i).  The folded A matrix is stored column-permuted
(per 256-block: even columns then odd columns) so the big GEMM's PSUM
partitions line up with the same pairing when its output chunks are used
as score-matmul weights.

Gate path: e = exp(scores/S) is transposed on the PE (identity matmul) so
den = e^T @ (-S*ones) and gs = e^T @ (head@u) become 1-column matmuls,
removing the partition-broadcast DRAM round-trip of hv entirely. The
sigmoid is evaluated as 1/(1+exp(-garg)) so the ACT engine only ever
needs the Exp table (Sigmoid lives in a different act-table set; each
switch would cost a 1.3us table reload); the sign flips ride host
constants (onesc=-S, mw=-S*mw, c0=-C0) and cancel in a_s/a_c. u and w2
are stored as fp8 hi+lo residual pairs - the gate dot-products were the
precision-dominant path (rel err 1.26e-2 -> 6.7e-3 on HW).
Scores carry a factor S=32 (folded into A) so the fp8 tensors sit inside
e4m3's dynamic range (max 240); exp applies scale 1/S.

Layout: batch is sharded 8 ways (512 batches/core); rows are processed in
tiles of 128 = 8 batches x 16 mentions. Cross-batch blocks are masked to
-inf via a rank-9 constant matmul and the per-tile key-mask via a rank-1
matmul, so softmax zeroes them exactly and the packed attention matrix is
block-diagonal -- which makes the pooling contractions plain matmuls
against block-diagonal weight columns built with a onehot multiply.
"""

import numpy as np
import ml_dtypes
import bass_rust
import concourse.bass as bass
import concourse.mybir as mybir
import concourse.tile as tile
from concourse.bass import ts
from concourse.bass_utils import run_bass_kernel_spmd

H = 768
B, M, N = 4096, 16, 16
NEG = -65504.0
P = 128
NCORES = 8
BC = B // NCORES          # batches per core = 512
ROWS = BC * M             # rows per core = 8192
TILES = ROWS // P         # 64 tiles (8 batches each)
ST = 4                    # tiles per supertile (GEMM moving N = 512)
NSUP = TILES // ST        # 16 supertiles
SN = ST * P               # 512 rows per supertile
GN = 512                  # GEMM moving width per PSUM pass (one bank)
KC = H // P               # 6 contraction chunks (128 each)
KC2 = H // (2 * P)        # 3 DoubleRow chunk pairs (256 each)
FO = 2 * H // P           # 12 score-feature chunks (A_s | A_c)
ACOLS = 2 * H + 1         # 1537: A_s | A_c | u
APAD = 1552               # ACOLS padded so the DoubleRow pair step is 16B-aligned
RPB = ROWS // M           # 512 pooled rows (batches) per core
S = 32.0                  # fp8 dynamic-range scale folded into A/u/w2/mw

F8 = mybir.dt.float8e4
F16 = mybir.dt.float16
F32 = mybir.dt.float32
U16 = mybir.dt.uint16
DR = mybir.MatmulPerfMode.DoubleRow
NP8 = ml_dtypes.float8_e4m3


def _split_sync_waits(nc):
    """Walrus caps sync waits per instruction (1 is the only universally
    accepted count in this toolchain). Hoist excess waits onto preceding
    single-wait EventSemaphore carriers on the same engine."""
    for f in nc.m.functions:
        for bb in f.blocks:
            il = bb.instructions
            new_il = []
            changed = False
            for inst in il:
                si = inst.sync_info
                if si is not None and len(si.on_wait) > 1:
                    waits = list(si.on_wait)
                    k = 0
                    while len(waits) > 1:
                        w, waits = waits[0], waits[1:]
                        d = bass_rust.InstEventSemaphore(
                            name=f"{inst.name}-wsplit{k}", ins=[], outs=[])
                        d.engine = inst.engine
                        d.sync_info = bass_rust.SyncInfo(on_wait=[w], on_update=[])
                        new_il.append(d)
                        k += 1
                        changed = True
                    inst.sync_info = bass_rust.SyncInfo(
                        on_wait=waits, on_update=list(si.on_update))
                new_il.append(inst)
            if changed:
                bb.instructions = new_il


def _build_nc(split=True):
    nc = bass.Bass(target_bir_lowering=False)

    head_d = nc.dram_tensor("head", [ROWS, H], F32, kind="ExternalInput")
    tail_d = nc.dram_tensor("tail", [ROWS, H], F32, kind="ExternalInput")
    acat_d = nc.dram_tensor("acat", [KC2, P, 2, APAD], F8, kind="ExternalInput")
    w2_d = nc.dram_tensor("w2c", [KC2, P, 2, 2], F8, kind="ExternalInput")
    wsvT_d = nc.dram_tensor("wsvT", [KC, P, H], F16, kind="ExternalInput")
    c9l_d = nc.dram_tensor("c9l", [9, P], F16, kind="ExternalInput")
    c9r_d = nc.dram_tensor("c9r", [9, P], F16, kind="ExternalInput")
    ones1_d = nc.dram_tensor("ones1", [1, P], F16, kind="ExternalInput")
    onesc_d = nc.dram_tensor("onesc", [P, 1], F16, kind="ExternalInput")
    onehot_d = nc.dram_tensor("onehot", [P, 8], F16, kind="ExternalInput")
    vs_d = nc.dram_tensor("vs", [1, TILES * P], F16, kind="ExternalInput")
    vc_d = nc.dram_tensor("vc", [1, TILES * P], F16, kind="ExternalInput")
    mw_d = nc.dram_tensor("mw", [P, TILES], F32, kind="ExternalInput")
    ident_d = nc.dram_tensor("ident", [P, P], F32, kind="ExternalInput")
    identh_d = nc.dram_tensor("identh", [P, P], F16, kind="ExternalInput")
    c0_d = nc.dram_tensor("c0", [P, 1], F32, kind="ExternalInput")
    out_d = nc.dram_tensor("out", [BC, H], F32, kind="ExternalOutput")

    with tile.TileContext(nc) as tc:
        _emit(nc, tc, head_d, tail_d, acat_d, w2_d, wsvT_d, c9l_d, c9r_d,
              ones1_d, onesc_d, onehot_d, vs_d, vc_d, mw_d, ident_d,
              identh_d, c0_d, out_d)
    if split:
        _split_sync_waits(nc)
    return nc


def _emit(nc, tc, head_d, tail_d, acat_d, w2_d, wsvT_d, c9l_d, c9r_d,
          ones1_d, onesc_d, onehot_d, vs_d, vc_d, mw_d, ident_d,
          identh_d, c0_d, out_d):
    from contextlib import ExitStack
    Exp = mybir.ActivationFunctionType.Exp
    Sig = mybir.ActivationFunctionType.Sigmoid
    Ident = mybir.ActivationFunctionType.Identity
    mult = mybir.AluOpType.mult
    ctx = ExitStack()
    with ctx:
        const = ctx.enter_context(tc.tile_pool(name="const", bufs=1))
        sup = ctx.enter_context(tc.tile_pool(name="sup", bufs=2))
        pt = ctx.enter_context(tc.tile_pool(name="pt", bufs=8))
        acc = ctx.enter_context(tc.tile_pool(name="acc", bufs=1))
        psg = ctx.enter_context(tc.tile_pool(name="psg", bufs=2, space="PSUM"))
        pss = ctx.enter_context(tc.tile_pool(name="pss", bufs=4, space="PSUM"))
        psw = ctx.enter_context(tc.tile_pool(name="psw", bufs=2, space="PSUM"))

        # ---- constants ----
        acat8 = const.tile([P, KC2, 2, APAD], F8)
        nc.sync.dma_start(out=acat8[:], in_=acat_d.rearrange("c p i m -> p c i m"))
        w2c8 = const.tile([P, KC2, 2, 2], F8)
        nc.sync.dma_start(out=w2c8[:], in_=w2_d.rearrange("c p i m -> p c i m"))
        c9l = const.tile([9, P], F16)
        nc.sync.dma_start(out=c9l[:], in_=c9l_d[:, :])
        c9r = const.tile([9, P], F16)
        nc.sync.dma_start(out=c9r[:], in_=c9r_d[:, :])
        ones1 = const.tile([1, P], F16)
        nc.sync.dma_start(out=ones1[:], in_=ones1_d[:, :])
        onesc = const.tile([P, 1], F16)
        nc.sync.dma_start(out=onesc[:], in_=onesc_d[:, :])
        onehot = const.tile([P, 8], F16)
        nc.sync.dma_start(out=onehot[:], in_=onehot_d[:, :])
        vs_all = const.tile([1, TILES * P], F16)
        nc.scalar.dma_start(out=vs_all[:], in_=vs_d[:, :])
        vc_all = const.tile([1, TILES * P], F16)
        nc.scalar.dma_start(out=vc_all[:], in_=vc_d[:, :])

        # ---- per-core accumulators ----
        hp_all = acc.tile([P, KC, RPB], F16)   # pooled head, feature-major
        tp_all = acc.tile([P, KC, RPB], F16)   # pooled tail, feature-major

        # loads are batched per PAIR of supertiles: the SWDGE descriptor-gen
        # time on the Pool engine is ~1-2.4us per instruction regardless of
        # size, and 4 cast-loads/supertile made Pool the DMA-issue serializer
        SG = ST
        GSUP = SG // ST
        head_r = head_d.rearrange("(g t p) h -> g p t h", t=SG, p=P)
        tail_r = tail_d.rearrange("(g t p) h -> g p t h", t=SG, p=P)

        loaded16 = {}
        loaded8 = {}

        def emit_loads16(g):
            # fp16 copies feed the value-side pools
            h16 = sup.tile([P, SG, H], F16, tag="h16", name=f"h16_{g}")
            t16 = sup.tile([P, SG, H], F16, tag="t16", name=f"t16_{g}")
            nc.gpsimd.dma_start(out=h16[:], in_=head_r[g])
            nc.gpsimd.dma_start(out=t16[:], in_=tail_r[g])
            loaded16[g] = (h16, t16)

        def emit_loads8(g):
            # fp8 copies feed the score-side GEMMs; cast directly from the
            # f32 rows by SWDGE
            h8 = sup.tile([P, SG, H], F8, tag="h8", name=f"h8_{g}")
            t8 = sup.tile([P, SG, H], F8, tag="t8", name=f"t8_{g}")
            nc.gpsimd.dma_start(out=h8[:], in_=head_r[g])
            nc.gpsimd.dma_start(out=t8[:], in_=tail_r[g])
            loaded8[g] = (h8, t8)

        transposed = {}

        def emit_transpose(s):
            # xbar transpose to feature-major with fp8 PAIRS packed in uint16
            h8, t8 = loaded8[s // GSUP]
            off = ST * (s % GSUP)
            hT = sup.tile([P, KC2, SN], U16, tag="hT", name=f"hT{s}", bufs=3)
            tT = sup.tile([P, KC2, SN], U16, tag="tT", name=f"tT{s}", bufs=3)
            for t in range(ST):
                nc.sync.dma_start_transpose(hT[:, :, ts(t, P)],
                                            h8[:, off + t, :].bitcast(U16))
                nc.sync.dma_start_transpose(tT[:, :, ts(t, P)],
                                            t8[:, off + t, :].bitcast(U16))
            transposed[s] = (hT, tT)

        def f8v(tT_, cc):
            # DoubleRow moving view of a pair-packed chunk: [K=128, 2, n]
            return tT_[:, cc, :].bitcast(F8).rearrange(
                "p (n two) -> p two n", two=2)

        out_fm = acc.tile([P, KC, RPB], F32)
        out_sb = acc.tile([P, BC // P, H], F32)
        out_r = out_d.rearrange("(r p) h -> p r h", p=P)

        def emit_final(half):
            # out = hpool @ Wsv^T + tpool for one half of the batches,
            # then transpose feature-major -> row-major and store.
            # Emitted per half so the first half overlaps the last supertile.
            bs = slice(half * (RPB // 2), (half + 1) * (RPB // 2))
            for j in range(KC):
                po_full = psg.tile([P, GN], F32, tag="pg", name=f"po{half}_{j}")
                po = po_full[:, :RPB // 2]
                for c in range(KC):
                    nc.tensor.matmul(po, wsvT[:, c, ts(j, P)], hp_all[:, c, bs],
                                     start=(c == 0), stop=(c == KC - 1))
                nc.vector.tensor_add(out=out_fm[:, j, bs], in0=po,
                                     in1=tp_all[:, j, bs])
            for r in range(half * (BC // P // 2), (half + 1) * (BC // P // 2)):
                for j in range(KC):
                    ptr_full = psg.tile([P, GN], F32, tag="pg", name=f"ptr{r}_{j}")
                    ptr = ptr_full[:, :P]
                    nc.tensor.transpose(ptr[:], out_fm[:, j, ts(r, P)], ident[:])
                    nc.scalar.copy(out_sb[:, r, ts(j, P)], ptr[:])
                nc.sync.dma_start(out=out_r[:, r, :], in_=out_sb[:, r, :])

        emit_loads16(0)
        emit_loads8(0)
        emit_transpose(0)
        wsvT = const.tile([P, KC, H], F16)
        nc.sync.dma_start(out=wsvT[:], in_=wsvT_d.rearrange("c p m -> p c m"))
        mw_all = const.tile([P, TILES], F32)
        nc.sync.dma_start(out=mw_all[:], in_=mw_d[:, :])
        ident = const.tile([P, P], F32)
        nc.sync.dma_start(out=ident[:], in_=ident_d[:, :])
        identh = const.tile([P, P], F16)
        nc.sync.dma_start(out=identh[:], in_=identh_d[:, :])
        c0 = const.tile([P, 1], F32)
        nc.sync.dma_start(out=c0[:], in_=c0_d[:, :])
        hA8s = {}

        def emit_gemm(s):
            # -- big GEMM: hA = head @ [A_s | A_c], feature-major, fp8 DR --
            hT, tT = transposed[s]
            hA8 = sup.tile([P, FO, SN], F8, tag="hA8", name=f"hA8_{s}")
            for j in range(FO):
                for hh in range(SN // GN):
                    pg = psg.tile([P, GN], F32, tag="pg")
                    for cc in range(KC2):
                        nc.tensor.matmul(pg[:], acat8[:, cc, :, ts(j, P)],
                                         f8v(hT, cc)[:, :, ts(hh, GN)],
                                         start=(cc == 0),
                                         stop=(cc == KC2 - 1), perf_mode=DR)
                    if (2 * j + hh) % 24 < 14:
                        nc.scalar.copy(hA8[:, j, ts(hh, GN)], pg[:])
                    else:
                        nc.vector.tensor_copy(hA8[:, j, ts(hh, GN)], pg[:])
            hA8s[s] = hA8

        def emit_tiles(s_idx):
            h16g, t16g = loaded16[s_idx // GSUP]
            voff = ST * (s_idx % GSUP)
            hT, tT = transposed.pop(s_idx)
            hA8 = hA8s.pop(s_idx)
            h16 = h16g[:, voff:voff + ST, :]
            t16 = t16g[:, voff:voff + ST, :]

            def tile_body(t, tg):
                # -- packed scores (8 batches x 16x16) + masks --
                ps_pair = pss.tile([P, 2, P], F32, tag="ps")
                ps_s = ps_pair[:, 0, :]
                ps_c = ps_pair[:, 1, :]
                for cc in range(KC2):
                    nc.tensor.matmul(ps_s, hA8[:, 2 * cc:2 * cc + 2, ts(t, P)],
                                     f8v(hT, cc)[:, :, ts(t, P)],
                                     start=(cc == 0), stop=False, perf_mode=DR)
                nc.tensor.matmul(ps_s, c9l[:], c9r[:], start=False, stop=False)
                nc.tensor.matmul(ps_s, ones1[:], vs_all[:, ts(tg, P)],
                                 start=False, stop=True)
                for cc in range(KC2):
                    nc.tensor.matmul(ps_c,
                                     hA8[:, KC + 2 * cc:KC + 2 * cc + 2, ts(t, P)],
                                     f8v(tT, cc)[:, :, ts(t, P)],
                                     start=(cc == 0), stop=False, perf_mode=DR)
                nc.tensor.matmul(ps_c, c9l[:], c9r[:], start=False, stop=False)
                nc.tensor.matmul(ps_c, ones1[:], vc_all[:, ts(tg, P)],
                                 start=False, stop=True)

                # -- gate dot inputs: hv = head@u, tv = tail@w2 (key-major,
                # 1-column DoubleRow matmuls, ~free on the PE) --
                # wp cols 104:232 (f32) double as the fp16 e^T landing zone
                # via bitcast, so eT shares wp's PSUM bank.
                wp = psw.tile([P, 232], F32, tag="wp")
                # u and w2 are kept as fp8 hi+lo pairs (the gate is the
                # precision-dominant path); the extra 1-column matmuls are
                # free on the PE (cost scales with out free size).
                for k in range(4 * KC2):
                    cc, i, r = k // 4, (k // 2) % 2, k % 2
                    nc.tensor.matmul(wp[:, 102:103],
                                     f8v(hT, cc)[:, i, ts(t, P)],
                                     acat8[:, cc, i, 1536 + r:1537 + r],
                                     start=(k == 0), stop=(k == 4 * KC2 - 1))
                for k in range(4 * KC2):
                    cc, i, r = k // 4, (k // 2) % 2, k % 2
                    nc.tensor.matmul(wp[:, 103:104],
                                     f8v(tT, cc)[:, i, ts(t, P)],
                                     w2c8[:, cc, i, r:r + 1],
                                     start=(k == 0), stop=(k == 4 * KC2 - 1))

                # -- softmax numerators (free axis); scores are O(5) bounded
                # and masked lanes are ~-2e3 after the 1/S exp scale, so fp32
                # exp neither overflows nor loses the reference's exactness --
                e_pair = pt.tile([P, 2, P], F16, tag="e_pair", bufs=6)
                nc.scalar.activation(out=e_pair[:], in_=ps_pair[:], func=Exp,
                                     bias=0.0, scale=1.0 / S)

                # -- e^T on the PE so den/gs become 1-column matmuls --
                eT = wp[:, 104:232].bitcast(F16).rearrange(
                    "p (c n) -> p c n", c=2)
                nc.tensor.transpose(eT[:, 0, :], e_pair[:, 0, :], identh[:])
                nc.tensor.transpose(eT[:, 1, :], e_pair[:, 1, :], identh[:])
                esT = pt.tile([P, 2, P], F16, tag="esT", bufs=6)
                nc.vector.tensor_copy(esT[:], eT[:])
                hvtv = pt.tile([P, 2], F16, tag="hvtv", bufs=4)
                nc.vector.tensor_copy(hvtv[:], wp[:, 102:104])

                # cols: 98 gs_num, 99 gc_num, 100 S*den_s, 101 S*den_c
                nc.tensor.matmul(wp[:, 98:99], esT[:, 0, :], hvtv[:, 0:1],
                                 start=True, stop=True)
                nc.tensor.matmul(wp[:, 99:100], esT[:, 1, :], hvtv[:, 1:2],
                                 start=True, stop=True)
                nc.tensor.matmul(wp[:, 100:101], esT[:, 0, :], onesc[:],
                                 start=True, stop=True)
                nc.tensor.matmul(wp[:, 101:102], esT[:, 1, :], onesc[:],
                                 start=True, stop=True)

                rden = pt.tile([P, 2], F32, tag="rden")
                nc.vector.reciprocal(out=rden[:], in_=wp[:, 100:102])
                # gate = sigmoid(garg) computed as 1/(1+exp(-garg)) so the ACT
                # engine only ever uses the Exp table (Sigmoid lives in a
                # different act-table set and every switch costs a 1.3us table
                # load). The sign flips ride the host constants: onesc=-S and
                # mw=-S*mw make rden negative and a_s/a_c positive again.
                t1 = pt.tile([P, 1], F32, tag="t1")
                nc.scalar.activation(out=t1[:], in_=wp[:, 99:100], func=Ident,
                                     bias=c0[:, 0:1], scale=rden[:, 1:2])
                eg = pt.tile([P, 1], F32, tag="eg")
                nc.scalar.activation(out=eg[:], in_=wp[:, 98:99], func=Exp,
                                     bias=t1[:, 0:1], scale=rden[:, 0:1])
                gp = pt.tile([P, 1], F32, tag="gp")
                nc.vector.tensor_scalar_add(out=gp[:], in0=eg[:], scalar1=1.0)
                gate = pt.tile([P, 1], F32, tag="gate")
                nc.vector.reciprocal(out=gate[:], in_=gp[:])

                # -- pooling coefficient vectors (fold S*mw and 1/(S den)) --
                mwg = pt.tile([P, 1], F16, tag="mwg")       # S*mw*gate
                nc.vector.tensor_mul(out=mwg[:], in0=mw_all[:, tg:tg + 1],
                                     in1=gate[:])
                a_s = pt.tile([P, 1], F16, tag="a_s")
                nc.vector.tensor_mul(out=a_s[:], in0=mwg[:], in1=rden[:, 0:1])
                mwc = pt.tile([P, 1], F16, tag="mwc")       # S*mw*(1-gate)
                nc.vector.tensor_sub(out=mwc[:], in0=mw_all[:, tg:tg + 1],
                                     in1=mwg[:])
                a_c = pt.tile([P, 1], F16, tag="a_c")
                nc.vector.tensor_mul(out=a_c[:], in0=mwc[:], in1=rden[:, 1:2])

                # -- ws = e^T @ a : per-key pooled weights (block-diag safe) --
                nc.tensor.matmul(wp[:, 96:97], e_pair[:, 0, :], a_s[:],
                                 start=True, stop=True)
                nc.tensor.matmul(wp[:, 97:98], e_pair[:, 1, :], a_c[:],
                                 start=True, stop=True)

                # -- block-diagonal weight columns via onehot --
                diag_s = pt.tile([P, 8], F16, tag="diag_s")
                diag_c = pt.tile([P, 8], F16, tag="diag_c")
                nc.vector.tensor_tensor(out=diag_s[:],
                                        in0=wp[:, 96:97].to_broadcast([P, 8]),
                                        in1=onehot[:], op=mult)
                nc.vector.tensor_tensor(out=diag_c[:],
                                        in0=wp[:, 97:98].to_broadcast([P, 8]),
                                        in1=onehot[:], op=mult)

                # -- pools: feature-major pooled vectors for 8 batches --
                ps_hp = wp[:, 0:48].rearrange("p (c e) -> p c e", e=8)
                ps_tp = wp[:, 48:96].rearrange("p (c e) -> p c e", e=8)
                for c in range(KC):
                    nc.tensor.matmul(ps_hp[:, c, :], h16[:, t, ts(c, P)],
                                     diag_s[:], start=True, stop=True)
                    nc.tensor.matmul(ps_tp[:, c, :], t16[:, t, ts(c, P)],
                                     diag_c[:], start=True, stop=True)
                nc.vector.tensor_copy(hp_all[:, :, tg * 8:(tg + 1) * 8], ps_hp)
                nc.vector.tensor_copy(tp_all[:, :, tg * 8:(tg + 1) * 8], ps_tp)

            for t in range(ST):
                tile_body(t, s_idx * ST + t)

        for s_idx in range(NSUP):
            if s_idx == NSUP // 2:
                emit_final(0)
            if s_idx % GSUP == 0 and s_idx // GSUP + 1 < NSUP // GSUP:
                emit_loads16(s_idx // GSUP + 1)
                emit_loads8(s_idx // GSUP + 1)
            if s_idx + 1 < NSUP:
                emit_transpose(s_idx + 1)
            emit_gemm(s_idx)
            emit_tiles(s_idx)

        emit_final(1)


_NC_CACHE = None


def _get_nc():
    global _NC_CACHE
    if _NC_CACHE is None:
        _NC_CACHE = _build_nc()
    return _NC_CACHE


def _host_prep(Wsq, Wsk, Wsv, Wcq, Wck, Wg, bg, bsv,
               head_mask, tail_mask):
    """Fold weights; build per-core constant tensors (shared across cores
    except the mask-derived ones)."""
    f64 = np.float64
    scale = 1.0 / np.sqrt(f64(H))
    A_s = (Wsq.astype(f64).T @ Wsk.astype(f64)) * scale
    A_c = (Wcq.astype(f64).T @ Wck.astype(f64)) * scale
    A = np.concatenate([A_s, A_c], axis=1)                         # [768, 1536]
    # per 256-block of output features: even columns then odd columns, so
    # the big GEMM's PSUM chunk pairs (2c, 2c+1) hold features 256c+2p+i
    colperm = np.concatenate([
        np.concatenate([np.arange(256 * b, 256 * b + 256, 2),
                        np.arange(256 * b + 1, 256 * b + 256, 2)])
        for b in range(2 * H // 256)])
    Wg1 = Wg[0, :H].astype(f64)
    w2 = Wg[0, H:].astype(f64)
    u = Wsv.astype(f64).T @ Wg1
    uS = S * u
    u_hi = (uS.astype(np.float32)).astype(NP8)
    u_lo = uS - u_hi.astype(f64)                  # quantized again by the cast
    w2S = S * w2
    w2_hi = (w2S.astype(np.float32)).astype(NP8)
    w2_lo = w2S - w2_hi.astype(f64)
    acat = np.concatenate([A[:, colperm], u_hi.astype(f64)[:, None] / S,
                           u_lo[:, None] / S,
                           np.zeros((H, APAD - ACOLS - 1))], axis=1)  # [768, 1552]
    # rows (input features) interleaved: acat8[c, p, i] = S*acat[256c+2p+i]
    acat8 = (S * acat).reshape(KC2, P, 2, APAD).astype(NP8)
    w2_8 = np.stack([w2_hi.astype(f64), w2_lo], axis=-1)
    w2_8 = (w2_8).reshape(KC2, P, 2, 2).astype(NP8)
    wsvT_t = Wsv.astype(f64).T.reshape(KC, P, H).astype(np.float16)

    g = np.arange(P) // M                                          # group id per row
    c9l = np.zeros((9, P), np.float16)
    c9r = np.zeros((9, P), np.float16)
    c9l[0] = 1.0
    c9r[0] = NEG
    for k in range(8):
        c9l[1 + k] = (g == k).astype(np.float16)
        c9r[1 + k] = -NEG * (g == k).astype(np.float16)
    ones1 = np.ones((1, P), np.float16)
    onesc = np.full((P, 1), -S, np.float16)   # negative: see gate-as-exp note
    onehot = np.zeros((P, 8), np.float16)
    onehot[np.arange(P), g] = 1.0

    C0 = float(bg[0] + f64(bsv) @ Wg1)
    c0 = np.full((P, 1), -C0, np.float32)     # negated: gate-as-exp
    ident = np.eye(P, dtype=np.float32)
    identh = np.eye(P, dtype=np.float16)

    # per-core mask-derived tensors
    hm = head_mask.reshape(NCORES, BC, M)
    tm = tail_mask.reshape(NCORES, BC, N)
    vs, vc, mw = [], [], []
    for i in range(NCORES):
        vs.append(((1 - hm[i]).astype(np.float16) * np.float16(NEG))
                  .reshape(1, TILES * P))
        vc.append(((1 - tm[i]).astype(np.float16) * np.float16(NEG))
                  .reshape(1, TILES * P))
        e = np.exp(hm[i].astype(f64))
        mwi = (-S * e / e.sum(axis=1, keepdims=True)).astype(np.float32)
        mw.append(mwi.reshape(TILES, P).T.copy())                    # [P, TILES]
    shared = dict(acat=acat8, w2c=w2_8, wsvT=wsvT_t, c9l=c9l, c9r=c9r,
                  ones1=ones1, onesc=onesc, onehot=onehot, ident=ident,
                  identh=identh, c0=c0)
    return shared, vs, vc, mw


def _core_feeds(head_mentions, tail_mentions, shared, vs, vc, mw, i):
    hm = head_mentions.reshape(NCORES, ROWS, H)
    tm = tail_mentions.reshape(NCORES, ROWS, H)
    feeds = {"head": np.ascontiguousarray(hm[i]),
             "tail": np.ascontiguousarray(tm[i]),
             "vs": vs[i], "vc": vc[i], "mw": mw[i]}
    feeds.update(shared)
    return feeds


def _reference_numpy(head_mentions, tail_mentions, head_mask, tail_mask,
                     Wsq, bsq, Wsk, bsk, Wsv, bsv, Wcq, bcq, Wck, bck, Wg, bg):
    """Exact fallback (only used if projection biases are nonzero)."""
    f = np.float32
    scale = f(1.0) / np.sqrt(f(H))
    hm = head_mentions.astype(f)
    tm = tail_mentions.astype(f)
    sq = hm @ Wsq.T + bsq
    sk = hm @ Wsk.T + bsk
    sv = hm @ Wsv.T + bsv
    ss = np.einsum("bmh,bnh->bmn", sq, sk) * scale
    ss = np.where(head_mask[:, None, :] == 0, f(NEG), ss)
    ss = ss - ss.max(-1, keepdims=True)
    e = np.exp(ss)
    sw = e / e.sum(-1, keepdims=True)
    self_out = np.einsum("bmn,bnh->bmh", sw, sv)
    cq = hm @ Wcq.T + bcq
    ck = tm @ Wck.T + bck
    cs = np.einsum("bmh,bnh->bmn", cq, ck) * scale
    cs = np.where(tail_mask[:, None, :] == 0, f(NEG), cs)
    cs = cs - cs.max(-1, keepdims=True)
    ec = np.exp(cs)
    cw = ec / ec.sum(-1, keepdims=True)
    cross_out = np.einsum("bmn,bnh->bmh", cw, tm)
    gate_in = np.concatenate([self_out, cross_out], axis=-1)
    gate = 1.0 / (1.0 + np.exp(-(np.einsum("bmh,oh->bmo", gate_in, Wg) + bg)))
    fused = gate * self_out + (1 - gate) * cross_out
    mexp = np.exp(head_mask.astype(f))
    mw = (mexp / mexp.sum(1, keepdims=True))[:, :, None]
    return (fused * mw).sum(axis=1)


def kernel(head_mentions, tail_mentions, head_mask, tail_mask,
           Wsq, bsq, Wsk, bsk, Wsv, bsv, Wcq, bcq, Wck, bck, Wg, bg,
           _trace=False):
    head_mentions = np.asarray(head_mentions)
    tail_mentions = np.asarray(tail_mentions)
    head_mask = np.asarray(head_mask)
    tail_mask = np.asarray(tail_mask)
    args = dict(Wsq=np.asarray(Wsq), bsq=np.asarray(bsq), Wsk=np.asarray(Wsk),
                bsk=np.asarray(bsk), Wsv=np.asarray(Wsv), bsv=np.asarray(bsv),
                Wcq=np.asarray(Wcq), bcq=np.asarray(bcq), Wck=np.asarray(Wck),
                bck=np.asarray(bck), Wg=np.asarray(Wg), bg=np.asarray(bg))

    # The folded formulation absorbs bg/bsv exactly; nonzero Q/K-side biases
    # (never produced by this problem's setup) would change the softmax and
    # are handled by the exact numpy fallback.
    if any(np.any(args[k] != 0) for k in ("bsq", "bsk", "bcq", "bck")):
        return _reference_numpy(head_mentions, tail_mentions, head_mask,
                                tail_mask, **args).astype(np.float32)

    shared, vs, vc, mw = _host_prep(args["Wsq"], args["Wsk"], args["Wsv"],
                                    args["Wcq"], args["Wck"], args["Wg"],
                                    args["bg"], args["bsv"],
                                    head_mask, tail_mask)

    nc = _get_nc()
    in_maps = [_core_feeds(head_mentions, tail_mentions, shared, vs, vc, mw, i)
               for i in range(NCORES)]
    res = run_bass_kernel_spmd(nc, in_maps, core_ids=list(range(NCORES)),
                               trace=_trace)
    out = np.concatenate([res.results[i]["out"] for i in range(NCORES)], axis=0)
    if _trace:
        kernel._last_result = res
    return out.astype(np.float32)


# revision 58
# speedup vs baseline: 1.0524x; 1.0472x over previous
"""Trainium2 Bass kernel for nn_EntityMentionAggregation.

Reference computation (per batch b, M=N=16 mentions, H=768):
  self-attn over head mentions, cross-attn head->tail, sigmoid-gated fusion,
  mask-softmax pooling over mentions -> out [B, H].

Algebraic restructuring (exact, given the zero biases produced by
setup_inputs; nonzero projection biases fall back to numpy):
  s_scores = scale * head @ (Wsq^T Wsk) @ head^T          (A_s folded)
  c_scores = scale * head @ (Wcq^T Wck) @ tail^T          (A_c folded)
  out      = hpool @ Wsv^T + tpool
    hpool  = ws_s^T-weighted sum of head rows, ws_s = s_w^T (mw*gate/den_s)
    tpool  = ws_c^T-weighted sum of tail rows
  gate     = sigmoid(s_w@(head@u) + c_w@(tail@w2) + C0), u = Wsv^T Wg1
so the V projection runs on pooled vectors (16x fewer rows) and
self_out/cross_out are never materialized.

Precision split: the score path (big GEMM + packed per-tile attention
matmuls + gate dot-products) runs in fp8 e4m3 with DoubleRow perf mode
(2 k-tiles of 128 per matmul at 0.5 cyc/row); the value path (pooled
head/tail rows, final Wsv^T projection) stays fp16. The fp8 operands are
produced by a second SWDGE cast-load (f32->fp8) and transposed to
feature-major via the SBUF xbar with PAIRS of fp8 values packed in one
uint16 element; the resulting [feat-pair partition, 2, row] layout is
exactly DoubleRow's expected [K,2,N] k-tile shape (logical feature
f = 256c + 2p + i).  The folded A matrix is stored column-permuted
(per 256-block: even columns then odd columns) so the big GEMM's PSUM
partitions line up with the same pairing when its output chunks are used
as score-matmul weights.

Gate path: e = exp(scores/S) is transposed on the PE (identity matmul) so
den = e^T @ (-S*ones) and gs = e^T @ (head@u) become 1-column matmuls,
removing the partition-broadcast DRAM round-trip of hv entirely. The
sigmoid is evaluated as 1/(1+exp(-garg)) so the ACT engine only ever
needs the Exp table (Sigmoid lives in a different act-table set; each
switch would cost a 1.3us table reload); the sign flips ride host
constants (onesc=-S, mw=-S*mw, c0=-C0) and cancel in a_s/a_c. u and w2
are stored as fp8 hi+lo residual pairs - the gate dot-products were the
precision-dominant path (rel err 1.26e-2 -> 6.7e-3 on HW).
Scores carry a factor S=32 (folded into A) so the fp8 tensors sit inside
e4m3's dynamic range (max 240); exp applies scale 1/S.

Layout: batch is sharded 8 ways (512 batches/core); rows are processed in
tiles of 128 = 8 batches x 16 mentions. Cross-batch blocks are masked to
-inf via a rank-9 constant matmul and the per-tile key-mask via a rank-1
matmul, so softmax zeroes them exactly and the packed attention matrix is
block-diagonal -- which makes the pooling contractions plain matmuls
against block-diagonal weight columns built with a onehot multiply.
"""

import numpy as np
import ml_dtypes
import bass_rust
import concourse.bass as bass
import concourse.mybir as mybir
import concourse.tile as tile
from concourse.bass import ts
from concourse.bass_utils import run_bass_kernel_spmd

H = 768
B, M, N = 4096, 16, 16
NEG = -65504.0
P = 128
NCORES = 8
BC = B // NCORES          # batches per core = 512
ROWS = BC * M             # rows per core = 8192
TILES = ROWS // P         # 64 tiles (8 batches each)
ST = 4                    # tiles per supertile (GEMM moving N = 512)
NSUP = TILES // ST        # 16 supertiles
SN = ST * P               # 512 rows per supertile
GN = 512                  # GEMM moving width per PSUM pass (one bank)
KC = H // P               # 6 contraction chunks (128 each)
KC2 = H // (2 * P)        # 3 DoubleRow chunk pairs (256 each)
FO = 2 * H // P           # 12 score-feature chunks (A_s | A_c)
ACOLS = 2 * H + 1         # 1537: A_s | A_c | u
APAD = 1552               # ACOLS padded so the DoubleRow pair step is 16B-aligned
RPB = ROWS // M           # 512 pooled rows (batches) per core
S = 32.0                  # fp8 dynamic-range scale folded into A/u/w2/mw

F8 = mybir.dt.float8e4
F16 = mybir.dt.float16
F32 = mybir.dt.float32
U16 = mybir.dt.uint16
DR = mybir.MatmulPerfMode.DoubleRow
NP8 = ml_dtypes.float8_e4m3


def _split_sync_waits(nc):
    """Walrus caps sync waits per instruction (1 is the only universally
    accepted count in this toolchain). Hoist excess waits onto preceding
    single-wait EventSemaphore carriers on the same engine."""
    for f in nc.m.functions:
        for bb in f.blocks:
            il = bb.instructions
            new_il = []
            changed = False
            for inst in il:
                si = inst.sync_info
                if si is not None and len(si.on_wait) > 1:
                    waits = list(si.on_wait)
                    k = 0
                    while len(waits) > 1:
                        w, waits = waits[0], waits[1:]
                        d = bass_rust.InstEventSemaphore(
                            name=f"{inst.name}-wsplit{k}", ins=[], outs=[])
                        d.engine = inst.engine
                        d.sync_info = bass_rust.SyncInfo(on_wait=[w], on_update=[])
                        new_il.append(d)
                        k += 1
                        changed = True
                    inst.sync_info = bass_rust.SyncInfo(
                        on_wait=waits, on_update=list(si.on_update))
                new_il.append(inst)
            if changed:
                bb.instructions = new_il


def _build_nc(split=True):
    nc = bass.Bass(target_bir_lowering=False)

    head_d = nc.dram_tensor("head", [ROWS, H], F32, kind="ExternalInput")
    tail_d = nc.dram_tensor("tail", [ROWS, H], F32, kind="ExternalInput")
    acat_d = nc.dram_tensor("acat", [KC2, P, 2, APAD], F8, kind="ExternalInput")
    w2_d = nc.dram_tensor("w2c", [KC2, P, 2, 2], F8, kind="ExternalInput")
    wsvT_d = nc.dram_tensor("wsvT", [KC, P, H], F16, kind="ExternalInput")
    c9l_d = nc.dram_tensor("c9l", [9, P], F16, kind="ExternalInput")
    c9r_d = nc.dram_tensor("c9r", [9, P], F16, kind="ExternalInput")
    ones1_d = nc.dram_tensor("ones1", [1, P], F16, kind="ExternalInput")
    onesc_d = nc.dram_tensor("onesc", [P, 1], F16, kind="ExternalInput")
    onehot_d = nc.dram_tensor("onehot", [P, 8], F16, kind="ExternalInput")
    vs_d = nc.dram_tensor("vs", [1, TILES * P], F16, kind="ExternalInput")
    vc_d = nc.dram_tensor("vc", [1, TILES * P], F16, kind="ExternalInput")
    mw_d = nc.dram_tensor("mw", [P, TILES], F32, kind="ExternalInput")
    ident_d = nc.dram_tensor("ident", [P, P], F32, kind="ExternalInput")
    identh_d = nc.dram_tensor("identh", [P, P], F16, kind="ExternalInput")
    c0_d = nc.dram_tensor("c0", [P, 1], F32, kind="ExternalInput")
    out_d = nc.dram_tensor("out", [BC, H], F32, kind="ExternalOutput")

    with tile.TileContext(nc) as tc:
        _emit(nc, tc, head_d, tail_d, acat_d, w2_d, wsvT_d, c9l_d, c9r_d,
              ones1_d, onesc_d, onehot_d, vs_d, vc_d, mw_d, ident_d,
              identh_d, c0_d, out_d)
    if split:
        _split_sync_waits(nc)
    return nc


def _emit(nc, tc, head_d, tail_d, acat_d, w2_d, wsvT_d, c9l_d, c9r_d,
          ones1_d, onesc_d, onehot_d, vs_d, vc_d, mw_d, ident_d,
          identh_d, c0_d, out_d):
    from contextlib import ExitStack
    Exp = mybir.ActivationFunctionType.Exp
    Sig = mybir.ActivationFunctionType.Sigmoid
    Ident = mybir.ActivationFunctionType.Identity
    mult = mybir.AluOpType.mult
    ctx = ExitStack()
    with ctx:
        const = ctx.enter_context(tc.tile_pool(name="const", bufs=1))
        sup = ctx.enter_context(tc.tile_pool(name="sup", bufs=2))
        pt = ctx.enter_context(tc.tile_pool(name="pt", bufs=8))
        acc = ctx.enter_context(tc.tile_pool(name="acc", bufs=1))
        psg = ctx.enter_context(tc.tile_pool(name="psg", bufs=2, space="PSUM"))
        pss = ctx.enter_context(tc.tile_pool(name="pss", bufs=4, space="PSUM"))
        psw = ctx.enter_context(tc.tile_pool(name="psw", bufs=2, space="PSUM"))

        # ---- constants ----
        acat8 = const.tile([P, KC2, 2, APAD], F8)
        nc.sync.dma_start(out=acat8[:], in_=acat_d.rearrange("c p i m -> p c i m"))
        w2c8 = const.tile([P, KC2, 2, 2], F8)
        nc.sync.dma_start(out=w2c8[:], in_=w2_d.rearrange("c p i m -> p c i m"))
        c9l = const.tile([9, P], F16)
        nc.sync.dma_start(out=c9l[:], in_=c9l_d[:, :])
        c9r = const.tile([9, P], F16)
        nc.sync.dma_start(out=c9r[:], in_=c9r_d[:, :])
        ones1 = const.tile([1, P], F16)
        nc.sync.dma_start(out=ones1[:], in_=ones1_d[:, :])
        onesc = const.tile([P, 1], F16)
        nc.sync.dma_start(out=onesc[:], in_=onesc_d[:, :])
        onehot = const.tile([P, 8], F16)
        nc.sync.dma_start(out=onehot[:], in_=onehot_d[:, :])
        vs_all = const.tile([1, TILES * P], F16)
        nc.scalar.dma_start(out=vs_all[:], in_=vs_d[:, :])
        vc_all = const.tile([1, TILES * P], F16)
        nc.scalar.dma_start(out=vc_all[:], in_=vc_d[:, :])

        # ---- per-core accumulators ----
        hp_all = acc.tile([P, KC, RPB], F16)   # pooled head, feature-major
        tp_all = acc.tile([P, KC, RPB], F16)   # pooled tail, feature-major

        # loads are batched per PAIR of supertiles: the SWDGE descriptor-gen
        # time on the Pool engine is ~1-2.4us per instruction regardless of
        # size, and 4 cast-loads/supertile made Pool the DMA-issue serializer
        SG = ST
        GSUP = SG // ST
        head_r = head_d.rearrange("(g t p) h -> g p t h", t=SG, p=P)
        tail_r = tail_d.rearrange("(g t p) h -> g p t h", t=SG, p=P)

        loaded16 = {}
        loaded8 = {}

        def emit_loads16(g):
            # fp16 copy feeds the TAIL value pools (tpool lands raw in the
            # output, fp8 there costs 3.5e-2 rel err); the HEAD value pools
            # read the fp8 copy instead -- hpool's quantization noise washes
            # through the Wsv^T projection (1.35e-2 total, still under gate)
            t16 = sup.tile([P, SG, H], F16, tag="t16", name=f"t16_{g}")
            nc.gpsimd.dma_start(out=t16[:], in_=tail_r[g])
            loaded16[g] = t16

        def emit_loads8(g):
            # fp8 copies feed the score-side GEMMs; cast directly from the
            # f32 rows by SWDGE
            h8 = sup.tile([P, SG, H], F8, tag="h8", name=f"h8_{g}", bufs=3)
            t8 = sup.tile([P, SG, H], F8, tag="t8", name=f"t8_{g}")
            nc.gpsimd.dma_start(out=h8[:], in_=head_r[g])
            nc.gpsimd.dma_start(out=t8[:], in_=tail_r[g])
            loaded8[g] = (h8, t8)

        transposed = {}

        def emit_transpose(s):
            # xbar transpose to feature-major with fp8 PAIRS packed in uint16
            h8, t8 = loaded8[s // GSUP]
            off = ST * (s % GSUP)
            hT = sup.tile([P, KC2, SN], U16, tag="hT", name=f"hT{s}", bufs=3)
            tT = sup.tile([P, KC2, SN], U16, tag="tT", name=f"tT{s}", bufs=3)
            for t in range(ST):
                nc.sync.dma_start_transpose(hT[:, :, ts(t, P)],
                                            h8[:, off + t, :].bitcast(U16))
                nc.sync.dma_start_transpose(tT[:, :, ts(t, P)],
                                            t8[:, off + t, :].bitcast(U16))
            transposed[s] = (hT, tT)

        def f8v(tT_, cc):
            # DoubleRow moving view of a pair-packed chunk: [K=128, 2, n]
            return tT_[:, cc, :].bitcast(F8).rearrange(
                "p (n two) -> p two n", two=2)

        out_fm = acc.tile([P, KC, RPB], F32)
        out_sb = acc.tile([P, BC // P, H], F32)
        out_r = out_d.rearrange("(r p) h -> p r h", p=P)

        def emit_final(half):
            # out = hpool @ Wsv^T + tpool for one half of the batches,
            # then transpose feature-major -> row-major and store.
            # Emitted per half so the first half overlaps the last supertile.
            bs = slice(half * (RPB // 2), (half + 1) * (RPB // 2))
            for j in range(KC):
                po_full = psg.tile([P, GN], F32, tag="pg", name=f"po{half}_{j}")
                po = po_full[:, :RPB // 2]
                for c in range(KC):
                    nc.tensor.matmul(po, wsvT[:, c, ts(j, P)], hp_all[:, c, bs],
                                     start=(c == 0), stop=(c == KC - 1))
                nc.vector.tensor_add(out=out_fm[:, j, bs], in0=po,
                                     in1=tp_all[:, j, bs])
            for r in range(half * (BC // P // 2), (half + 1) * (BC // P // 2)):
                for j in range(KC):
                    ptr_full = psg.tile([P, GN], F32, tag="pg", name=f"ptr{r}_{j}")
                    ptr = ptr_full[:, :P]
                    nc.tensor.transpose(ptr[:], out_fm[:, j, ts(r, P)], ident[:])
                    nc.scalar.copy(out_sb[:, r, ts(j, P)], ptr[:])
                nc.sync.dma_start(out=out_r[:, r, :], in_=out_sb[:, r, :])

        emit_loads16(0)
        emit_loads8(0)
        emit_transpose(0)
        wsvT = const.tile([P, KC, H], F16)
        nc.sync.dma_start(out=wsvT[:], in_=wsvT_d.rearrange("c p m -> p c m"))
        mw_all = const.tile([P, TILES], F32)
        nc.sync.dma_start(out=mw_all[:], in_=mw_d[:, :])
        ident = const.tile([P, P], F32)
        nc.sync.dma_start(out=ident[:], in_=ident_d[:, :])
        identh = const.tile([P, P], F16)
        nc.sync.dma_start(out=identh[:], in_=identh_d[:, :])
        c0 = const.tile([P, 1], F32)
        nc.sync.dma_start(out=c0[:], in_=c0_d[:, :])
        hA8s = {}

        def emit_gemm(s):
            # -- big GEMM: hA = head @ [A_s | A_c], feature-major, fp8 DR --
            hT, tT = transposed[s]
            hA8 = sup.tile([P, FO, SN], F8, tag="hA8", name=f"hA8_{s}")
            for j in range(FO):
                for hh in range(SN // GN):
                    pg = psg.tile([P, GN], F32, tag="pg")
                    for cc in range(KC2):
                        nc.tensor.matmul(pg[:], acat8[:, cc, :, ts(j, P)],
                                         f8v(hT, cc)[:, :, ts(hh, GN)],
                                         start=(cc == 0),
                                         stop=(cc == KC2 - 1), perf_mode=DR)
                    if (2 * j + hh) % 24 < 14:
                        nc.scalar.copy(hA8[:, j, ts(hh, GN)], pg[:])
                    else:
                        nc.vector.tensor_copy(hA8[:, j, ts(hh, GN)], pg[:])
            hA8s[s] = hA8

        def emit_tiles(s_idx):
            t16g = loaded16[s_idx // GSUP]
            h8g, _t8g = loaded8[s_idx // GSUP]
            voff = ST * (s_idx % GSUP)
            hT, tT = transposed.pop(s_idx)
            hA8 = hA8s.pop(s_idx)
            h16 = h8g[:, voff:voff + ST, :]
            t16 = t16g[:, voff:voff + ST, :]

            def tile_body(t, tg):
                # -- packed scores (8 batches x 16x16) + masks --
                ps_pair = pss.tile([P, 2, P], F32, tag="ps")
                ps_s = ps_pair[:, 0, :]
                ps_c = ps_pair[:, 1, :]
                for cc in range(KC2):
                    nc.tensor.matmul(ps_s, hA8[:, 2 * cc:2 * cc + 2, ts(t, P)],
                                     f8v(hT, cc)[:, :, ts(t, P)],
                                     start=(cc == 0), stop=False, perf_mode=DR)
                nc.tensor.matmul(ps_s, c9l[:], c9r[:], start=False, stop=False)
                nc.tensor.matmul(ps_s, ones1[:], vs_all[:, ts(tg, P)],
                                 start=False, stop=True)
                for cc in range(KC2):
                    nc.tensor.matmul(ps_c,
                                     hA8[:, KC + 2 * cc:KC + 2 * cc + 2, ts(t, P)],
                                     f8v(tT, cc)[:, :, ts(t, P)],
                                     start=(cc == 0), stop=False, perf_mode=DR)
                nc.tensor.matmul(ps_c, c9l[:], c9r[:], start=False, stop=False)
                nc.tensor.matmul(ps_c, ones1[:], vc_all[:, ts(tg, P)],
                                 start=False, stop=True)

                # -- gate dot inputs: hv = head@u, tv = tail@w2 (key-major,
                # 1-column DoubleRow matmuls, ~free on the PE) --
                # wp cols 104:232 (f32) double as the fp16 e^T landing zone
                # via bitcast, so eT shares wp's PSUM bank.
                wp = psw.tile([P, 232], F32, tag="wp")
                # u and w2 are kept as fp8 hi+lo pairs (the gate is the
                # precision-dominant path); the extra 1-column matmuls are
                # free on the PE (cost scales with out free size).
                for k in range(4 * KC2):
                    cc, i, r = k // 4, (k // 2) % 2, k % 2
                    nc.tensor.matmul(wp[:, 102:103],
                                     f8v(hT, cc)[:, i, ts(t, P)],
                                     acat8[:, cc, i, 1536 + r:1537 + r],
                                     start=(k == 0), stop=(k == 4 * KC2 - 1))
                for k in range(4 * KC2):
                    cc, i, r = k // 4, (k // 2) % 2, k % 2
                    nc.tensor.matmul(wp[:, 103:104],
                                     f8v(tT, cc)[:, i, ts(t, P)],
                                     w2c8[:, cc, i, r:r + 1],
                                     start=(k == 0), stop=(k == 4 * KC2 - 1))

                # -- softmax numerators (free axis); scores are O(5) bounded
                # and masked lanes are ~-2e3 after the 1/S exp scale, so fp32
                # exp neither overflows nor loses the reference's exactness --
                e_pair = pt.tile([P, 2, P], F16, tag="e_pair", bufs=6)
                nc.scalar.activation(out=e_pair[:], in_=ps_pair[:], func=Exp,
                                     bias=0.0, scale=1.0 / S)

                # -- e^T on the PE so den/gs become 1-column matmuls --
                eT = wp[:, 104:232].bitcast(F16).rearrange(
                    "p (c n) -> p c n", c=2)
                nc.tensor.transpose(eT[:, 0, :], e_pair[:, 0, :], identh[:])
                nc.tensor.transpose(eT[:, 1, :], e_pair[:, 1, :], identh[:])
                esT = pt.tile([P, 2, P], F16, tag="esT", bufs=6)
                nc.vector.tensor_copy(esT[:], eT[:])
                hvtv = pt.tile([P, 2], F16, tag="hvtv", bufs=4)
                nc.vector.tensor_copy(hvtv[:], wp[:, 102:104])

                # cols: 98 gs_num, 99 gc_num, 100 S*den_s, 101 S*den_c
                nc.tensor.matmul(wp[:, 98:99], esT[:, 0, :], hvtv[:, 0:1],
                                 start=True, stop=True)
                nc.tensor.matmul(wp[:, 99:100], esT[:, 1, :], hvtv[:, 1:2],
                                 start=True, stop=True)
                nc.tensor.matmul(wp[:, 100:101], esT[:, 0, :], onesc[:],
                                 start=True, stop=True)
                nc.tensor.matmul(wp[:, 101:102], esT[:, 1, :], onesc[:],
                                 start=True, stop=True)

                rden = pt.tile([P, 2], F32, tag="rden")
                nc.vector.reciprocal(out=rden[:], in_=wp[:, 100:102])
                # gate = sigmoid(garg) computed as 1/(1+exp(-garg)) so the ACT
                # engine only ever uses the Exp table (Sigmoid lives in a
                # different act-table set and every switch costs a 1.3us table
                # load). The sign flips ride the host constants: onesc=-S and
                # mw=-S*mw make rden negative and a_s/a_c positive again.
                t1 = pt.tile([P, 1], F32, tag="t1")
                nc.scalar.activation(out=t1[:], in_=wp[:, 99:100], func=Ident,
                                     bias=c0[:, 0:1], scale=rden[:, 1:2])
                eg = pt.tile([P, 1], F32, tag="eg")
                nc.scalar.activation(out=eg[:], in_=wp[:, 98:99], func=Exp,
                                     bias=t1[:, 0:1], scale=rden[:, 0:1])
                gp = pt.tile([P, 1], F32, tag="gp")
                nc.vector.tensor_scalar_add(out=gp[:], in0=eg[:], scalar1=1.0)
                gate = pt.tile([P, 1], F32, tag="gate")
                nc.vector.reciprocal(out=gate[:], in_=gp[:])

                # -- pooling coefficient vectors (fold S*mw and 1/(S den)) --
                mwg = pt.tile([P, 1], F16, tag="mwg")       # S*mw*gate
                nc.vector.tensor_mul(out=mwg[:], in0=mw_all[:, tg:tg + 1],
                                     in1=gate[:])
                a_s = pt.tile([P, 1], F16, tag="a_s")
                nc.vector.tensor_mul(out=a_s[:], in0=mwg[:], in1=rden[:, 0:1])
                mwc = pt.tile([P, 1], F16, tag="mwc")       # S*mw*(1-gate)
                nc.vector.tensor_sub(out=mwc[:], in0=mw_all[:, tg:tg + 1],
                                     in1=mwg[:])
                a_c = pt.tile([P, 1], F16, tag="a_c")
                nc.vector.tensor_mul(out=a_c[:], in0=mwc[:], in1=rden[:, 1:2])

                # -- ws = e^T @ a : per-key pooled weights (block-diag safe) --
                nc.tensor.matmul(wp[:, 96:97], e_pair[:, 0, :], a_s[:],
                                 start=True, stop=True)
                nc.tensor.matmul(wp[:, 97:98], e_pair[:, 1, :], a_c[:],
                                 start=True, stop=True)

                # -- block-diagonal weight columns via onehot --
                diag_s = pt.tile([P, 8], F16, tag="diag_s")
                diag_c = pt.tile([P, 8], F16, tag="diag_c")
                nc.vector.tensor_tensor(out=diag_s[:],
                                        in0=wp[:, 96:97].to_broadcast([P, 8]),
                                        in1=onehot[:], op=mult)
                nc.vector.tensor_tensor(out=diag_c[:],
                                        in0=wp[:, 97:98].to_broadcast([P, 8]),
                                        in1=onehot[:], op=mult)

                # -- pools: feature-major pooled vectors for 8 batches --
                ps_hp = wp[:, 0:48].rearrange("p (c e) -> p c e", e=8)
                ps_tp = wp[:, 48:96].rearrange("p (c e) -> p c e", e=8)
                for c in range(KC):
                    nc.tensor.matmul(ps_hp[:, c, :], h16[:, t, ts(c, P)],
                                     diag_s[:], start=True, stop=True)
                    nc.tensor.matmul(ps_tp[:, c, :], t16[:, t, ts(c, P)],
                                     diag_c[:], start=True, stop=True)
                nc.vector.tensor_copy(hp_all[:, :, tg * 8:(tg + 1) * 8], ps_hp)
                nc.vector.tensor_copy(tp_all[:, :, tg * 8:(tg + 1) * 8], ps_tp)

            for t in range(ST):
                tile_body(t, s_idx * ST + t)

        for s_idx in range(NSUP):
            if s_idx == NSUP // 2:
                emit_final(0)
            if s_idx % GSUP == 0 and s_idx // GSUP + 1 < NSUP // GSUP:
                emit_loads16(s_idx // GSUP + 1)
                emit_loads8(s_idx // GSUP + 1)
            if s_idx + 1 < NSUP:
                emit_transpose(s_idx + 1)
            emit_gemm(s_idx)
            emit_tiles(s_idx)

        emit_final(1)


_NC_CACHE = None


def _get_nc():
    global _NC_CACHE
    if _NC_CACHE is None:
        _NC_CACHE = _build_nc()
    return _NC_CACHE


def _host_prep(Wsq, Wsk, Wsv, Wcq, Wck, Wg, bg, bsv,
               head_mask, tail_mask):
    """Fold weights; build per-core constant tensors (shared across cores
    except the mask-derived ones)."""
    f64 = np.float64
    scale = 1.0 / np.sqrt(f64(H))
    A_s = (Wsq.astype(f64).T @ Wsk.astype(f64)) * scale
    A_c = (Wcq.astype(f64).T @ Wck.astype(f64)) * scale
    A = np.concatenate([A_s, A_c], axis=1)                         # [768, 1536]
    # per 256-block of output features: even columns then odd columns, so
    # the big GEMM's PSUM chunk pairs (2c, 2c+1) hold features 256c+2p+i
    colperm = np.concatenate([
        np.concatenate([np.arange(256 * b, 256 * b + 256, 2),
                        np.arange(256 * b + 1, 256 * b + 256, 2)])
        for b in range(2 * H // 256)])
    Wg1 = Wg[0, :H].astype(f64)
    w2 = Wg[0, H:].astype(f64)
    u = Wsv.astype(f64).T @ Wg1
    uS = S * u
    u_hi = (uS.astype(np.float32)).astype(NP8)
    u_lo = uS - u_hi.astype(f64)                  # quantized again by the cast
    w2S = S * w2
    w2_hi = (w2S.astype(np.float32)).astype(NP8)
    w2_lo = w2S - w2_hi.astype(f64)
    acat = np.concatenate([A[:, colperm], u_hi.astype(f64)[:, None] / S,
                           u_lo[:, None] / S,
                           np.zeros((H, APAD - ACOLS - 1))], axis=1)  # [768, 1552]
    # rows (input features) interleaved: acat8[c, p, i] = S*acat[256c+2p+i]
    acat8 = (S * acat).reshape(KC2, P, 2, APAD).astype(NP8)
    w2_8 = np.stack([w2_hi.astype(f64), w2_lo], axis=-1)
    w2_8 = (w2_8).reshape(KC2, P, 2, 2).astype(NP8)
    wsvT_t = Wsv.astype(f64).T.reshape(KC, P, H).astype(np.float16)

    g = np.arange(P) // M                                          # group id per row
    c9l = np.zeros((9, P), np.float16)
    c9r = np.zeros((9, P), np.float16)
    c9l[0] = 1.0
    c9r[0] = NEG
    for k in range(8):
        c9l[1 + k] = (g == k).astype(np.float16)
        c9r[1 + k] = -NEG * (g == k).astype(np.float16)
    ones1 = np.ones((1, P), np.float16)
    onesc = np.full((P, 1), -S, np.float16)   # negative: see gate-as-exp note
    onehot = np.zeros((P, 8), np.float16)
    onehot[np.arange(P), g] = 1.0

    C0 = float(bg[0] + f64(bsv) @ Wg1)
    c0 = np.full((P, 1), -C0, np.float32)     # negated: gate-as-exp
    ident = np.eye(P, dtype=np.float32)
    identh = np.eye(P, dtype=np.float16)

    # per-core mask-derived tensors
    hm = head_mask.reshape(NCORES, BC, M)
    tm = tail_mask.reshape(NCORES, BC, N)
    vs, vc, mw = [], [], []
    for i in range(NCORES):
        vs.append(((1 - hm[i]).astype(np.float16) * np.float16(NEG))
                  .reshape(1, TILES * P))
        vc.append(((1 - tm[i]).astype(np.float16) * np.float16(NEG))
                  .reshape(1, TILES * P))
        e = np.exp(hm[i].astype(f64))
        mwi = (-S * e / e.sum(axis=1, keepdims=True)).astype(np.float32)
        mw.append(mwi.reshape(TILES, P).T.copy())                    # [P, TILES]
    shared = dict(acat=acat8, w2c=w2_8, wsvT=wsvT_t, c9l=c9l, c9r=c9r,
                  ones1=ones1, onesc=onesc, onehot=onehot, ident=ident,
                  identh=identh, c0=c0)
    return shared, vs, vc, mw


def _core_feeds(head_mentions, tail_mentions, shared, vs, vc, mw, i):
    hm = head_mentions.reshape(NCORES, ROWS, H)
    tm = tail_mentions.reshape(NCORES, ROWS, H)
    feeds = {"head": np.ascontiguousarray(hm[i]),
             "tail": np.ascontiguousarray(tm[i]),
             "vs": vs[i], "vc": vc[i], "mw": mw[i]}
    feeds.update(shared)
    return feeds


def _reference_numpy(head_mentions, tail_mentions, head_mask, tail_mask,
                     Wsq, bsq, Wsk, bsk, Wsv, bsv, Wcq, bcq, Wck, bck, Wg, bg):
    """Exact fallback (only used if projection biases are nonzero)."""
    f = np.float32
    scale = f(1.0) / np.sqrt(f(H))
    hm = head_mentions.astype(f)
    tm = tail_mentions.astype(f)
    sq = hm @ Wsq.T + bsq
    sk = hm @ Wsk.T + bsk
    sv = hm @ Wsv.T + bsv
    ss = np.einsum("bmh,bnh->bmn", sq, sk) * scale
    ss = np.where(head_mask[:, None, :] == 0, f(NEG), ss)
    ss = ss - ss.max(-1, keepdims=True)
    e = np.exp(ss)
    sw = e / e.sum(-1, keepdims=True)
    self_out = np.einsum("bmn,bnh->bmh", sw, sv)
    cq = hm @ Wcq.T + bcq
    ck = tm @ Wck.T + bck
    cs = np.einsum("bmh,bnh->bmn", cq, ck) * scale
    cs = np.where(tail_mask[:, None, :] == 0, f(NEG), cs)
    cs = cs - cs.max(-1, keepdims=True)
    ec = np.exp(cs)
    cw = ec / ec.sum(-1, keepdims=True)
    cross_out = np.einsum("bmn,bnh->bmh", cw, tm)
    gate_in = np.concatenate([self_out, cross_out], axis=-1)
    gate = 1.0 / (1.0 + np.exp(-(np.einsum("bmh,oh->bmo", gate_in, Wg) + bg)))
    fused = gate * self_out + (1 - gate) * cross_out
    mexp = np.exp(head_mask.astype(f))
    mw = (mexp / mexp.sum(1, keepdims=True))[:, :, None]
    return (fused * mw).sum(axis=1)


def kernel(head_mentions, tail_mentions, head_mask, tail_mask,
           Wsq, bsq, Wsk, bsk, Wsv, bsv, Wcq, bcq, Wck, bck, Wg, bg,
           _trace=False):
    head_mentions = np.asarray(head_mentions)
    tail_mentions = np.asarray(tail_mentions)
    head_mask = np.asarray(head_mask)
    tail_mask = np.asarray(tail_mask)
    args = dict(Wsq=np.asarray(Wsq), bsq=np.asarray(bsq), Wsk=np.asarray(Wsk),
                bsk=np.asarray(bsk), Wsv=np.asarray(Wsv), bsv=np.asarray(bsv),
                Wcq=np.asarray(Wcq), bcq=np.asarray(bcq), Wck=np.asarray(Wck),
                bck=np.asarray(bck), Wg=np.asarray(Wg), bg=np.asarray(bg))

    # The folded formulation absorbs bg/bsv exactly; nonzero Q/K-side biases
    # (never produced by this problem's setup) would change the softmax and
    # are handled by the exact numpy fallback.
    if any(np.any(args[k] != 0) for k in ("bsq", "bsk", "bcq", "bck")):
        return _reference_numpy(head_mentions, tail_mentions, head_mask,
                                tail_mask, **args).astype(np.float32)

    shared, vs, vc, mw = _host_prep(args["Wsq"], args["Wsk"], args["Wsv"],
                                    args["Wcq"], args["Wck"], args["Wg"],
                                    args["bg"], args["bsv"],
                                    head_mask, tail_mask)

    nc = _get_nc()
    in_maps = [_core_feeds(head_mentions, tail_mentions, shared, vs, vc, mw, i)
               for i in range(NCORES)]
    res = run_bass_kernel_spmd(nc, in_maps, core_ids=list(range(NCORES)),
                               trace=_trace)
    out = np.concatenate([res.results[i]["out"] for i in range(NCORES)], axis=0)
    if _trace:
        kernel._last_result = res
    return out.astype(np.float32)


# revision 59
# speedup vs baseline: 1.0536x; 1.0011x over previous
"""Trainium2 Bass kernel for nn_EntityMentionAggregation.

Reference computation (per batch b, M=N=16 mentions, H=768):
  self-attn over head mentions, cross-attn head->tail, sigmoid-gated fusion,
  mask-softmax pooling over mentions -> out [B, H].

Algebraic restructuring (exact, given the zero biases produced by
setup_inputs; nonzero projection biases fall back to numpy):
  s_scores = scale * head @ (Wsq^T Wsk) @ head^T          (A_s folded)
  c_scores = scale * head @ (Wcq^T Wck) @ tail^T          (A_c folded)
  out      = hpool @ Wsv^T + tpool
    hpool  = ws_s^T-weighted sum of head rows, ws_s = s_w^T (mw*gate/den_s)
    tpool  = ws_c^T-weighted sum of tail rows
  gate     = sigmoid(s_w@(head@u) + c_w@(tail@w2) + C0), u = Wsv^T Wg1
so the V projection runs on pooled vectors (16x fewer rows) and
self_out/cross_out are never materialized.

Precision split: the score path (big GEMM + packed per-tile attention
matmuls + gate dot-products) runs in fp8 e4m3 with DoubleRow perf mode
(2 k-tiles of 128 per matmul at 0.5 cyc/row); the value path (pooled
head/tail rows, final Wsv^T projection) stays fp16. The fp8 operands are
produced by a second SWDGE cast-load (f32->fp8) and transposed to
feature-major via the SBUF xbar with PAIRS of fp8 values packed in one
uint16 element; the resulting [feat-pair partition, 2, row] layout is
exactly DoubleRow's expected [K,2,N] k-tile shape (logical feature
f = 256c + 2p + i).  The folded A matrix is stored column-permuted
(per 256-block: even columns then odd columns) so the big GEMM's PSUM
partitions line up with the same pairing when its output chunks are used
as score-matmul weights.

Gate path: e = exp(scores/S) is transposed on the PE (identity matmul) so
den = e^T @ (-S*ones) and gs = e^T @ (head@u) become 1-column matmuls,
removing the partition-broadcast DRAM round-trip of hv entirely. The
sigmoid is evaluated as 1/(1+exp(-garg)) so the ACT engine only ever
needs the Exp table (Sigmoid lives in a different act-table set; each
switch would cost a 1.3us table reload); the sign flips ride host
constants (onesc=-S, mw=-S*mw, c0=-C0) and cancel in a_s/a_c. u and w2
are stored as fp8 hi+lo residual pairs - the gate dot-products were the
precision-dominant path (rel err 1.26e-2 -> 6.7e-3 on HW).
Scores carry a factor S=32 (folded into A) so the fp8 tensors sit inside
e4m3's dynamic range (max 240); exp applies scale 1/S.

Layout: batch is sharded 8 ways (512 batches/core); rows are processed in
tiles of 128 = 8 batches x 16 mentions. Cross-batch blocks are masked to
-inf via a rank-9 constant matmul and the per-tile key-mask via a rank-1
matmul, so softmax zeroes them exactly and the packed attention matrix is
block-diagonal -- which makes the pooling contractions plain matmuls
against block-diagonal weight columns built with a onehot multiply.
"""

import numpy as np
import ml_dtypes
import bass_rust
import concourse.bass as bass
import concourse.mybir as mybir
import concourse.tile as tile
from concourse.bass import ts
from concourse.bass_utils import run_bass_kernel_spmd

H = 768
B, M, N = 4096, 16, 16
NEG = -65504.0
P = 128
NCORES = 8
BC = B // NCORES          # batches per core = 512
ROWS = BC * M             # rows per core = 8192
TILES = ROWS // P         # 64 tiles (8 batches each)
ST = 4                    # tiles per supertile (GEMM moving N = 512)
NSUP = TILES // ST        # 16 supertiles
SN = ST * P               # 512 rows per supertile
GN = 512                  # GEMM moving width per PSUM pass (one bank)
KC = H // P               # 6 contraction chunks (128 each)
KC2 = H // (2 * P)        # 3 DoubleRow chunk pairs (256 each)
FO = 2 * H // P           # 12 score-feature chunks (A_s | A_c)
ACOLS = 2 * H + 1         # 1537: A_s | A_c | u
APAD = 1552               # ACOLS padded so the DoubleRow pair step is 16B-aligned
RPB = ROWS // M           # 512 pooled rows (batches) per core
S = 32.0                  # fp8 dynamic-range scale folded into A/u/w2/mw

F8 = mybir.dt.float8e4
F16 = mybir.dt.float16
F32 = mybir.dt.float32
U16 = mybir.dt.uint16
DR = mybir.MatmulPerfMode.DoubleRow
NP8 = ml_dtypes.float8_e4m3


def _split_sync_waits(nc):
    """Walrus caps sync waits per instruction (1 is the only universally
    accepted count in this toolchain). Hoist excess waits onto preceding
    single-wait EventSemaphore carriers on the same engine."""
    for f in nc.m.functions:
        for bb in f.blocks:
            il = bb.instructions
            new_il = []
            changed = False
            for inst in il:
                si = inst.sync_info
                if si is not None and len(si.on_wait) > 1:
                    waits = list(si.on_wait)
                    k = 0
                    while len(waits) > 1:
                        w, waits = waits[0], waits[1:]
                        d = bass_rust.InstEventSemaphore(
                            name=f"{inst.name}-wsplit{k}", ins=[], outs=[])
                        d.engine = inst.engine
                        d.sync_info = bass_rust.SyncInfo(on_wait=[w], on_update=[])
                        new_il.append(d)
                        k += 1
                        changed = True
                    inst.sync_info = bass_rust.SyncInfo(
                        on_wait=waits, on_update=list(si.on_update))
                new_il.append(inst)
            if changed:
                bb.instructions = new_il


def _build_nc(split=True):
    nc = bass.Bass(target_bir_lowering=False)

    head_d = nc.dram_tensor("head", [ROWS, H], F32, kind="ExternalInput")
    tail_d = nc.dram_tensor("tail", [ROWS, H], F32, kind="ExternalInput")
    acat_d = nc.dram_tensor("acat", [KC2, P, 2, APAD], F8, kind="ExternalInput")
    w2_d = nc.dram_tensor("w2c", [KC2, P, 2, 2], F8, kind="ExternalInput")
    wsvT_d = nc.dram_tensor("wsvT", [KC, P, H], F16, kind="ExternalInput")
    c9l_d = nc.dram_tensor("c9l", [9, P], F16, kind="ExternalInput")
    c9r_d = nc.dram_tensor("c9r", [9, P], F16, kind="ExternalInput")
    ones1_d = nc.dram_tensor("ones1", [1, P], F16, kind="ExternalInput")
    onesc_d = nc.dram_tensor("onesc", [P, 1], F16, kind="ExternalInput")
    onehot_d = nc.dram_tensor("onehot", [P, 8], F16, kind="ExternalInput")
    vs_d = nc.dram_tensor("vs", [1, TILES * P], F16, kind="ExternalInput")
    vc_d = nc.dram_tensor("vc", [1, TILES * P], F16, kind="ExternalInput")
    mw_d = nc.dram_tensor("mw", [P, TILES], F32, kind="ExternalInput")
    ident_d = nc.dram_tensor("ident", [P, P], F32, kind="ExternalInput")
    identh_d = nc.dram_tensor("identh", [P, P], F16, kind="ExternalInput")
    c0_d = nc.dram_tensor("c0", [P, 1], F32, kind="ExternalInput")
    out_d = nc.dram_tensor("out", [BC, H], F32, kind="ExternalOutput")

    with tile.TileContext(nc) as tc:
        _emit(nc, tc, head_d, tail_d, acat_d, w2_d, wsvT_d, c9l_d, c9r_d,
              ones1_d, onesc_d, onehot_d, vs_d, vc_d, mw_d, ident_d,
              identh_d, c0_d, out_d)
    if split:
        _split_sync_waits(nc)
    return nc


def _emit(nc, tc, head_d, tail_d, acat_d, w2_d, wsvT_d, c9l_d, c9r_d,
          ones1_d, onesc_d, onehot_d, vs_d, vc_d, mw_d, ident_d,
          identh_d, c0_d, out_d):
    from contextlib import ExitStack
    Exp = mybir.ActivationFunctionType.Exp
    Sig = mybir.ActivationFunctionType.Sigmoid
    Ident = mybir.ActivationFunctionType.Identity
    mult = mybir.AluOpType.mult
    ctx = ExitStack()
    with ctx:
        const = ctx.enter_context(tc.tile_pool(name="const", bufs=1))
        sup = ctx.enter_context(tc.tile_pool(name="sup", bufs=2))
        pt = ctx.enter_context(tc.tile_pool(name="pt", bufs=8))
        acc = ctx.enter_context(tc.tile_pool(name="acc", bufs=1))
        psg = ctx.enter_context(tc.tile_pool(name="psg", bufs=2, space="PSUM"))
        pss = ctx.enter_context(tc.tile_pool(name="pss", bufs=3, space="PSUM"))
        psw = ctx.enter_context(tc.tile_pool(name="psw", bufs=3, space="PSUM"))

        # ---- constants ----
        acat8 = const.tile([P, KC2, 2, APAD], F8)
        nc.sync.dma_start(out=acat8[:], in_=acat_d.rearrange("c p i m -> p c i m"))
        w2c8 = const.tile([P, KC2, 2, 2], F8)
        nc.sync.dma_start(out=w2c8[:], in_=w2_d.rearrange("c p i m -> p c i m"))
        c9l = const.tile([9, P], F16)
        nc.sync.dma_start(out=c9l[:], in_=c9l_d[:, :])
        c9r = const.tile([9, P], F16)
        nc.sync.dma_start(out=c9r[:], in_=c9r_d[:, :])
        ones1 = const.tile([1, P], F16)
        nc.sync.dma_start(out=ones1[:], in_=ones1_d[:, :])
        onesc = const.tile([P, 1], F16)
        nc.sync.dma_start(out=onesc[:], in_=onesc_d[:, :])
        onehot = const.tile([P, 8], F16)
        nc.sync.dma_start(out=onehot[:], in_=onehot_d[:, :])
        vs_all = const.tile([1, TILES * P], F16)
        nc.scalar.dma_start(out=vs_all[:], in_=vs_d[:, :])
        vc_all = const.tile([1, TILES * P], F16)
        nc.scalar.dma_start(out=vc_all[:], in_=vc_d[:, :])

        # ---- per-core accumulators ----
        hp_all = acc.tile([P, KC, RPB], F16)   # pooled head, feature-major
        tp_all = acc.tile([P, KC, RPB], F16)   # pooled tail, feature-major

        # loads are batched per PAIR of supertiles: the SWDGE descriptor-gen
        # time on the Pool engine is ~1-2.4us per instruction regardless of
        # size, and 4 cast-loads/supertile made Pool the DMA-issue serializer
        SG = ST
        GSUP = SG // ST
        head_r = head_d.rearrange("(g t p) h -> g p t h", t=SG, p=P)
        tail_r = tail_d.rearrange("(g t p) h -> g p t h", t=SG, p=P)

        loaded16 = {}
        loaded8 = {}

        def emit_loads16(g):
            # fp16 copy feeds the TAIL value pools (tpool lands raw in the
            # output, fp8 there costs 3.5e-2 rel err); the HEAD value pools
            # read the fp8 copy instead -- hpool's quantization noise washes
            # through the Wsv^T projection (1.35e-2 total, still under gate)
            t16 = sup.tile([P, SG, H], F16, tag="t16", name=f"t16_{g}")
            nc.gpsimd.dma_start(out=t16[:], in_=tail_r[g])
            loaded16[g] = t16

        def emit_loads8(g):
            # fp8 copies feed the score-side GEMMs; cast directly from the
            # f32 rows by SWDGE
            h8 = sup.tile([P, SG, H], F8, tag="h8", name=f"h8_{g}", bufs=3)
            t8 = sup.tile([P, SG, H], F8, tag="t8", name=f"t8_{g}")
            nc.gpsimd.dma_start(out=h8[:], in_=head_r[g])
            nc.gpsimd.dma_start(out=t8[:], in_=tail_r[g])
            loaded8[g] = (h8, t8)

        transposed = {}

        def emit_transpose(s):
            # xbar transpose to feature-major with fp8 PAIRS packed in uint16
            h8, t8 = loaded8[s // GSUP]
            off = ST * (s % GSUP)
            hT = sup.tile([P, KC2, SN], U16, tag="hT", name=f"hT{s}", bufs=3)
            tT = sup.tile([P, KC2, SN], U16, tag="tT", name=f"tT{s}", bufs=3)
            for t in range(ST):
                nc.sync.dma_start_transpose(hT[:, :, ts(t, P)],
                                            h8[:, off + t, :].bitcast(U16))
                nc.sync.dma_start_transpose(tT[:, :, ts(t, P)],
                                            t8[:, off + t, :].bitcast(U16))
            transposed[s] = (hT, tT)

        def f8v(tT_, cc):
            # DoubleRow moving view of a pair-packed chunk: [K=128, 2, n]
            return tT_[:, cc, :].bitcast(F8).rearrange(
                "p (n two) -> p two n", two=2)

        out_fm = acc.tile([P, KC, RPB], F32)
        out_sb = acc.tile([P, BC // P, H], F32)
        out_r = out_d.rearrange("(r p) h -> p r h", p=P)

        def emit_final(half):
            # out = hpool @ Wsv^T + tpool for one half of the batches,
            # then transpose feature-major -> row-major and store.
            # Emitted per half so the first half overlaps the last supertile.
            bs = slice(half * (RPB // 2), (half + 1) * (RPB // 2))
            for j in range(KC):
                po_full = psg.tile([P, GN], F32, tag="pg", name=f"po{half}_{j}")
                po = po_full[:, :RPB // 2]
                for c in range(KC):
                    nc.tensor.matmul(po, wsvT[:, c, ts(j, P)], hp_all[:, c, bs],
                                     start=(c == 0), stop=(c == KC - 1))
                nc.vector.tensor_add(out=out_fm[:, j, bs], in0=po,
                                     in1=tp_all[:, j, bs])
            for r in range(half * (BC // P // 2), (half + 1) * (BC // P // 2)):
                for j in range(KC):
                    ptr_full = psg.tile([P, GN], F32, tag="pg", name=f"ptr{r}_{j}")
                    ptr = ptr_full[:, :P]
                    nc.tensor.transpose(ptr[:], out_fm[:, j, ts(r, P)], ident[:])
                    nc.scalar.copy(out_sb[:, r, ts(j, P)], ptr[:])
                nc.sync.dma_start(out=out_r[:, r, :], in_=out_sb[:, r, :])

        emit_loads16(0)
        emit_loads8(0)
        emit_transpose(0)
        wsvT = const.tile([P, KC, H], F16)
        nc.sync.dma_start(out=wsvT[:], in_=wsvT_d.rearrange("c p m -> p c m"))
        mw_all = const.tile([P, TILES], F32)
        nc.sync.dma_start(out=mw_all[:], in_=mw_d[:, :])
        ident = const.tile([P, P], F32)
        nc.sync.dma_start(out=ident[:], in_=ident_d[:, :])
        identh = const.tile([P, P], F16)
        nc.sync.dma_start(out=identh[:], in_=identh_d[:, :])
        c0 = const.tile([P, 1], F32)
        nc.sync.dma_start(out=c0[:], in_=c0_d[:, :])
        hA8s = {}

        def emit_gemm(s):
            # -- big GEMM: hA = head @ [A_s | A_c], feature-major, fp8 DR --
            hT, tT = transposed[s]
            hA8 = sup.tile([P, FO, SN], F8, tag="hA8", name=f"hA8_{s}")
            for j in range(FO):
                for hh in range(SN // GN):
                    pg = psg.tile([P, GN], F32, tag="pg")
                    for cc in range(KC2):
                        nc.tensor.matmul(pg[:], acat8[:, cc, :, ts(j, P)],
                                         f8v(hT, cc)[:, :, ts(hh, GN)],
                                         start=(cc == 0),
                                         stop=(cc == KC2 - 1), perf_mode=DR)
                    if (2 * j + hh) % 24 < 14:
                        nc.scalar.copy(hA8[:, j, ts(hh, GN)], pg[:])
                    else:
                        nc.vector.tensor_copy(hA8[:, j, ts(hh, GN)], pg[:])
            hA8s[s] = hA8

        def emit_tiles(s_idx):
            t16g = loaded16[s_idx // GSUP]
            h8g, _t8g = loaded8[s_idx // GSUP]
            voff = ST * (s_idx % GSUP)
            hT, tT = transposed.pop(s_idx)
            hA8 = hA8s.pop(s_idx)
            h16 = h8g[:, voff:voff + ST, :]
            t16 = t16g[:, voff:voff + ST, :]

            def tile_body(t, tg):
                # -- packed scores (8 batches x 16x16) + masks --
                ps_pair = pss.tile([P, 2, P], F32, tag="ps")
                ps_s = ps_pair[:, 0, :]
                ps_c = ps_pair[:, 1, :]
                for cc in range(KC2):
                    nc.tensor.matmul(ps_s, hA8[:, 2 * cc:2 * cc + 2, ts(t, P)],
                                     f8v(hT, cc)[:, :, ts(t, P)],
                                     start=(cc == 0), stop=False, perf_mode=DR)
                nc.tensor.matmul(ps_s, c9l[:], c9r[:], start=False, stop=False)
                nc.tensor.matmul(ps_s, ones1[:], vs_all[:, ts(tg, P)],
                                 start=False, stop=True)
                for cc in range(KC2):
                    nc.tensor.matmul(ps_c,
                                     hA8[:, KC + 2 * cc:KC + 2 * cc + 2, ts(t, P)],
                                     f8v(tT, cc)[:, :, ts(t, P)],
                                     start=(cc == 0), stop=False, perf_mode=DR)
                nc.tensor.matmul(ps_c, c9l[:], c9r[:], start=False, stop=False)
                nc.tensor.matmul(ps_c, ones1[:], vc_all[:, ts(tg, P)],
                                 start=False, stop=True)

                # -- gate dot inputs: hv = head@u, tv = tail@w2 (key-major,
                # 1-column DoubleRow matmuls, ~free on the PE) --
                # wp cols 104:232 (f32) double as the fp16 e^T landing zone
                # via bitcast, so eT shares wp's PSUM bank.
                wp = psw.tile([P, 232], F32, tag="wp")
                # u and w2 are kept as fp8 hi+lo pairs (the gate is the
                # precision-dominant path); the extra 1-column matmuls are
                # free on the PE (cost scales with out free size).
                for k in range(4 * KC2):
                    cc, i, r = k // 4, (k // 2) % 2, k % 2
                    nc.tensor.matmul(wp[:, 102:103],
                                     f8v(hT, cc)[:, i, ts(t, P)],
                                     acat8[:, cc, i, 1536 + r:1537 + r],
                                     start=(k == 0), stop=(k == 4 * KC2 - 1))
                for k in range(4 * KC2):
                    cc, i, r = k // 4, (k // 2) % 2, k % 2
                    nc.tensor.matmul(wp[:, 103:104],
                                     f8v(tT, cc)[:, i, ts(t, P)],
                                     w2c8[:, cc, i, r:r + 1],
                                     start=(k == 0), stop=(k == 4 * KC2 - 1))

                # -- softmax numerators (free axis); scores are O(5) bounded
                # and masked lanes are ~-2e3 after the 1/S exp scale, so fp32
                # exp neither overflows nor loses the reference's exactness --
                e_pair = pt.tile([P, 2, P], F16, tag="e_pair", bufs=6)
                nc.scalar.activation(out=e_pair[:], in_=ps_pair[:], func=Exp,
                                     bias=0.0, scale=1.0 / S)

                # -- e^T on the PE so den/gs become 1-column matmuls --
                eT = wp[:, 104:232].bitcast(F16).rearrange(
                    "p (c n) -> p c n", c=2)
                nc.tensor.transpose(eT[:, 0, :], e_pair[:, 0, :], identh[:])
                nc.tensor.transpose(eT[:, 1, :], e_pair[:, 1, :], identh[:])
                esT = pt.tile([P, 2, P], F16, tag="esT", bufs=6)
                nc.vector.tensor_copy(esT[:], eT[:])
                hvtv = pt.tile([P, 2], F16, tag="hvtv", bufs=4)
                nc.vector.tensor_copy(hvtv[:], wp[:, 102:104])

                # cols: 98 gs_num, 99 gc_num, 100 S*den_s, 101 S*den_c
                nc.tensor.matmul(wp[:, 98:99], esT[:, 0, :], hvtv[:, 0:1],
                                 start=True, stop=True)
                nc.tensor.matmul(wp[:, 99:100], esT[:, 1, :], hvtv[:, 1:2],
                                 start=True, stop=True)
                nc.tensor.matmul(wp[:, 100:101], esT[:, 0, :], onesc[:],
                                 start=True, stop=True)
                nc.tensor.matmul(wp[:, 101:102], esT[:, 1, :], onesc[:],
                                 start=True, stop=True)

                rden = pt.tile([P, 2], F32, tag="rden")
                nc.vector.reciprocal(out=rden[:], in_=wp[:, 100:102])
                # gate = sigmoid(garg) computed as 1/(1+exp(-garg)) so the ACT
                # engine only ever uses the Exp table (Sigmoid lives in a
                # different act-table set and every switch costs a 1.3us table
                # load). The sign flips ride the host constants: onesc=-S and
                # mw=-S*mw make rden negative and a_s/a_c positive again.
                t1 = pt.tile([P, 1], F32, tag="t1")
                nc.scalar.activation(out=t1[:], in_=wp[:, 99:100], func=Ident,
                                     bias=c0[:, 0:1], scale=rden[:, 1:2])
                eg = pt.tile([P, 1], F32, tag="eg")
                nc.scalar.activation(out=eg[:], in_=wp[:, 98:99], func=Exp,
                                     bias=t1[:, 0:1], scale=rden[:, 0:1])
                gp = pt.tile([P, 1], F32, tag="gp")
                nc.vector.tensor_scalar_add(out=gp[:], in0=eg[:], scalar1=1.0)
                gate = pt.tile([P, 1], F32, tag="gate")
                nc.vector.reciprocal(out=gate[:], in_=gp[:])

                # -- pooling coefficient vectors (fold S*mw and 1/(S den)) --
                mwg = pt.tile([P, 1], F16, tag="mwg")       # S*mw*gate
                nc.vector.tensor_mul(out=mwg[:], in0=mw_all[:, tg:tg + 1],
                                     in1=gate[:])
                a_s = pt.tile([P, 1], F16, tag="a_s")
                nc.vector.tensor_mul(out=a_s[:], in0=mwg[:], in1=rden[:, 0:1])
                mwc = pt.tile([P, 1], F16, tag="mwc")       # S*mw*(1-gate)
                nc.vector.tensor_sub(out=mwc[:], in0=mw_all[:, tg:tg + 1],
                                     in1=mwg[:])
                a_c = pt.tile([P, 1], F16, tag="a_c")
                nc.vector.tensor_mul(out=a_c[:], in0=mwc[:], in1=rden[:, 1:2])

                # -- ws = e^T @ a : per-key pooled weights (block-diag safe) --
                nc.tensor.matmul(wp[:, 96:97], e_pair[:, 0, :], a_s[:],
                                 start=True, stop=True)
                nc.tensor.matmul(wp[:, 97:98], e_pair[:, 1, :], a_c[:],
                                 start=True, stop=True)

                # -- block-diagonal weight columns via onehot --
                diag_s = pt.tile([P, 8], F16, tag="diag_s")
                diag_c = pt.tile([P, 8], F16, tag="diag_c")
                nc.vector.tensor_tensor(out=diag_s[:],
                                        in0=wp[:, 96:97].to_broadcast([P, 8]),
                                        in1=onehot[:], op=mult)
                nc.vector.tensor_tensor(out=diag_c[:],
                                        in0=wp[:, 97:98].to_broadcast([P, 8]),
                                        in1=onehot[:], op=mult)

                # -- pools: feature-major pooled vectors for 8 batches --
                ps_hp = wp[:, 0:48].rearrange("p (c e) -> p c e", e=8)
                ps_tp = wp[:, 48:96].rearrange("p (c e) -> p c e", e=8)
                for c in range(KC):
                    nc.tensor.matmul(ps_hp[:, c, :], h16[:, t, ts(c, P)],
                                     diag_s[:], start=True, stop=True)
                    nc.tensor.matmul(ps_tp[:, c, :], t16[:, t, ts(c, P)],
                                     diag_c[:], start=True, stop=True)
                nc.vector.tensor_copy(hp_all[:, :, tg * 8:(tg + 1) * 8], ps_hp)
                nc.vector.tensor_copy(tp_all[:, :, tg * 8:(tg + 1) * 8], ps_tp)

            for t in range(ST):
                tile_body(t, s_idx * ST + t)

        for s_idx in range(NSUP):
            if s_idx == NSUP // 2:
                emit_final(0)
            if s_idx % GSUP == 0 and s_idx // GSUP + 1 < NSUP // GSUP:
                emit_loads16(s_idx // GSUP + 1)
                emit_loads8(s_idx // GSUP + 1)
            if s_idx + 1 < NSUP:
                emit_transpose(s_idx + 1)
            emit_gemm(s_idx)
            emit_tiles(s_idx)

        emit_final(1)


_NC_CACHE = None


def _get_nc():
    global _NC_CACHE
    if _NC_CACHE is None:
        _NC_CACHE = _build_nc()
    return _NC_CACHE


def _host_prep(Wsq, Wsk, Wsv, Wcq, Wck, Wg, bg, bsv,
               head_mask, tail_mask):
    """Fold weights; build per-core constant tensors (shared across cores
    except the mask-derived ones)."""
    f64 = np.float64
    scale = 1.0 / np.sqrt(f64(H))
    A_s = (Wsq.astype(f64).T @ Wsk.astype(f64)) * scale
    A_c = (Wcq.astype(f64).T @ Wck.astype(f64)) * scale
    A = np.concatenate([A_s, A_c], axis=1)                         # [768, 1536]
    # per 256-block of output features: even columns then odd columns, so
    # the big GEMM's PSUM chunk pairs (2c, 2c+1) hold features 256c+2p+i
    colperm = np.concatenate([
        np.concatenate([np.arange(256 * b, 256 * b + 256, 2),
                        np.arange(256 * b + 1, 256 * b + 256, 2)])
        for b in range(2 * H // 256)])
    Wg1 = Wg[0, :H].astype(f64)
    w2 = Wg[0, H:].astype(f64)
    u = Wsv.astype(f64).T @ Wg1
    uS = S * u
    u_hi = (uS.astype(np.float32)).astype(NP8)
    u_lo = uS - u_hi.astype(f64)                  # quantized again by the cast
    w2S = S * w2
    w2_hi = (w2S.astype(np.float32)).astype(NP8)
    w2_lo = w2S - w2_hi.astype(f64)
    acat = np.concatenate([A[:, colperm], u_hi.astype(f64)[:, None] / S,
                           u_lo[:, None] / S,
                           np.zeros((H, APAD - ACOLS - 1))], axis=1)  # [768, 1552]
    # rows (input features) interleaved: acat8[c, p, i] = S*acat[256c+2p+i]
    acat8 = (S * acat).reshape(KC2, P, 2, APAD).astype(NP8)
    w2_8 = np.stack([w2_hi.astype(f64), w2_lo], axis=-1)
    w2_8 = (w2_8).reshape(KC2, P, 2, 2).astype(NP8)
    wsvT_t = Wsv.astype(f64).T.reshape(KC, P, H).astype(np.float16)

    g = np.arange(P) // M                                          # group id per row
    c9l = np.zeros((9, P), np.float16)
    c9r = np.zeros((9, P), np.float16)
    c9l[0] = 1.0
    c9r[0] = NEG
    for k in range(8):
        c9l[1 + k] = (g == k).astype(np.float16)
        c9r[1 + k] = -NEG * (g == k).astype(np.float16)
    ones1 = np.ones((1, P), np.float16)
    onesc = np.full((P, 1), -S, np.float16)   # negative: see gate-as-exp note
    onehot = np.zeros((P, 8), np.float16)
    onehot[np.arange(P), g] = 1.0

    C0 = float(bg[0] + f64(bsv) @ Wg1)
    c0 = np.full((P, 1), -C0, np.float32)     # negated: gate-as-exp
    ident = np.eye(P, dtype=np.float32)
    identh = np.eye(P, dtype=np.float16)

    # per-core mask-derived tensors
    hm = head_mask.reshape(NCORES, BC, M)
    tm = tail_mask.reshape(NCORES, BC, N)
    vs, vc, mw = [], [], []
    for i in range(NCORES):
        vs.append(((1 - hm[i]).astype(np.float16) * np.float16(NEG))
                  .reshape(1, TILES * P))
        vc.append(((1 - tm[i]).astype(np.float16) * np.float16(NEG))
                  .reshape(1, TILES * P))
        e = np.exp(hm[i].astype(f64))
        mwi = (-S * e / e.sum(axis=1, keepdims=True)).astype(np.float32)
        mw.append(mwi.reshape(TILES, P).T.copy())                    # [P, TILES]
    shared = dict(acat=acat8, w2c=w2_8, wsvT=wsvT_t, c9l=c9l, c9r=c9r,
                  ones1=ones1, onesc=onesc, onehot=onehot, ident=ident,
                  identh=identh, c0=c0)
    return shared, vs, vc, mw


def _core_feeds(head_mentions, tail_mentions, shared, vs, vc, mw, i):
    hm = head_mentions.reshape(NCORES, ROWS, H)
    tm = tail_mentions.reshape(NCORES, ROWS, H)
    feeds = {"head": np.ascontiguousarray(hm[i]),
             "tail": np.ascontiguousarray(tm[i]),
             "vs": vs[i], "vc": vc[i], "mw": mw[i]}
    feeds.update(shared)
    return feeds


def _reference_numpy(head_mentions, tail_mentions, head_mask, tail_mask,
                     Wsq, bsq, Wsk, bsk, Wsv, bsv, Wcq, bcq, Wck, bck, Wg, bg):
    """Exact fallback (only used if projection biases are nonzero)."""
    f = np.float32
    scale = f(1.0) / np.sqrt(f(H))
    hm = head_mentions.astype(f)
    tm = tail_mentions.astype(f)
    sq = hm @ Wsq.T + bsq
    sk = hm @ Wsk.T + bsk
    sv = hm @ Wsv.T + bsv
    ss = np.einsum("bmh,bnh->bmn", sq, sk) * scale
    ss = np.where(head_mask[:, None, :] == 0, f(NEG), ss)
    ss = ss - ss.max(-1, keepdims=True)
    e = np.exp(ss)
    sw = e / e.sum(-1, keepdims=True)
    self_out = np.einsum("bmn,bnh->bmh", sw, sv)
    cq = hm @ Wcq.T + bcq
    ck = tm @ Wck.T + bck
    cs = np.einsum("bmh,bnh->bmn", cq, ck) * scale
    cs = np.where(tail_mask[:, None, :] == 0, f(NEG), cs)
    cs = cs - cs.max(-1, keepdims=True)
    ec = np.exp(cs)
    cw = ec / ec.sum(-1, keepdims=True)
    cross_out = np.einsum("bmn,bnh->bmh", cw, tm)
    gate_in = np.concatenate([self_out, cross_out], axis=-1)
    gate = 1.0 / (1.0 + np.exp(-(np.einsum("bmh,oh->bmo", gate_in, Wg) + bg)))
    fused = gate * self_out + (1 - gate) * cross_out
    mexp = np.exp(head_mask.astype(f))
    mw = (mexp / mexp.sum(1, keepdims=True))[:, :, None]
    return (fused * mw).sum(axis=1)


def kernel(head_mentions, tail_mentions, head_mask, tail_mask,
           Wsq, bsq, Wsk, bsk, Wsv, bsv, Wcq, bcq, Wck, bck, Wg, bg,
           _trace=False):
    head_mentions = np.asarray(head_mentions)
    tail_mentions = np.asarray(tail_mentions)
    head_mask = np.asarray(head_mask)
    tail_mask = np.asarray(tail_mask)
    args = dict(Wsq=np.asarray(Wsq), bsq=np.asarray(bsq), Wsk=np.asarray(Wsk),
                bsk=np.asarray(bsk), Wsv=np.asarray(Wsv), bsv=np.asarray(bsv),
                Wcq=np.asarray(Wcq), bcq=np.asarray(bcq), Wck=np.asarray(Wck),
                bck=np.asarray(bck), Wg=np.asarray(Wg), bg=np.asarray(bg))

    # The folded formulation absorbs bg/bsv exactly; nonzero Q/K-side biases
    # (never produced by this problem's setup) would change the softmax and
    # are handled by the exact numpy fallback.
    if any(np.any(args[k] != 0) for k in ("bsq", "bsk", "bcq", "bck")):
        return _reference_numpy(head_mentions, tail_mentions, head_mask,
                                tail_mask, **args).astype(np.float32)

    shared, vs, vc, mw = _host_prep(args["Wsq"], args["Wsk"], args["Wsv"],
                                    args["Wcq"], args["Wck"], args["Wg"],
                                    args["bg"], args["bsv"],
                                    head_mask, tail_mask)

    nc = _get_nc()
    in_maps = [_core_feeds(head_mentions, tail_mentions, shared, vs, vc, mw, i)
               for i in range(NCORES)]
    res = run_bass_kernel_spmd(nc, in_maps, core_ids=list(range(NCORES)),
                               trace=_trace)
    out = np.concatenate([res.results[i]["out"] for i in range(NCORES)], axis=0)
    if _trace:
        kernel._last_result = res
    return out.astype(np.float32)
